# revision 2
# baseline (speedup 1.0000x reference)
"""Trainium2 Bass kernel for an attention MPNN layer (edge MLP + segment
softmax + scatter-mean + node MLP), distributed over 8 NeuronCores.

Strategy: host sorts edges by destination node and partitions BOTH the nodes
and their incoming edges across the 8 cores (node range [c*N/8,(c+1)*N/8) and
every edge pointing into it live on core c). Each core is then fully
independent -- no collectives. Within a core, nodes are processed in tiles of
128; a tile's incoming edges stream through the edge MLPs in chunks of 128,
and the segment softmax/mean reduction is realised as a mask matmul
(mask[e, n] = p_e * [dst_e == n]) accumulated in PSUM across the tile's
chunks. All irregularity (gather of nf[src]/nf[dst], sort, ragged segment
boundaries) is resolved on the host, so the device only executes dense
DMAs and matmuls.
"""

import math

import numpy as np

D = 64
H = 128
NCORES = 8
PT = 128  # nodes per tile (partition dim)
CHUNK = 128  # edges per sub-chunk
GB = 4  # sub-chunks per h-stage group (moving-dim 512)

# dtype of the stage-1 stream (edge features + layer-1 weights).
STAGE1_BF16 = False

_PROG_CACHE = {}


# --------------------------------------------------------------------------
# host-side preprocessing
# --------------------------------------------------------------------------

def _prep(nf, ef, src, dst):
    N, E = nf.shape[0], ef.shape[0]
    NPER = -(-N // NCORES)
    NT = -(-NPER // PT)

    src = np.ascontiguousarray(src).astype(np.int64, copy=False)
    dst = np.ascontiguousarray(dst).astype(np.int64, copy=False)

    perm = np.argsort(dst, kind="stable")
    dsts = dst[perm]
    srcs = src[perm]
    core_e = dsts // NPER
    loc = dsts - core_e * NPER
    tloc = loc // PT
    gid = core_e * NT + tloc  # nondecreasing
    starts = np.searchsorted(gid, np.arange(NCORES * NT + 1), side="left")
    K = np.diff(starts).reshape(NCORES, NT)

    n_ch = np.maximum(1, -(-K.max(axis=0) // CHUNK)).astype(np.int64)  # [NT]
    offs = np.zeros(NT + 1, np.int64)
    offs[1:] = np.cumsum(n_ch) * CHUNK
    E_pad = int(offs[-1])
    MAXJ = int(n_ch.max())

    efs = ef[perm]
    nfss = nf[srcs]
    nfds = nf[dsts]
    deg = np.bincount(dst, minlength=N).astype(np.float64)
    invdeg = (1.0 / np.maximum(deg, 1.0)).astype(np.float32)

    A1 = np.zeros((NCORES, 2 * D, E_pad), np.float32)
    A2 = np.zeros((NCORES, D, E_pad), np.float32)
    DL = np.full((NCORES, NT, PT, MAXJ + 1), -1.0, np.float32)
    NFT = np.zeros((NCORES, D, NT * PT), np.float32)

    for c in range(NCORES):
        for t in range(NT):
            g = c * NT + t
            s0, cnt = starts[g], K[c, t]
            o = offs[t]
            if cnt:
                A1[c, :D, o:o + cnt] = efs[s0:s0 + cnt].T
                A1[c, D:, o:o + cnt] = nfss[s0:s0 + cnt].T
                A2[c, :, o:o + cnt] = nfds[s0:s0 + cnt].T
                pad = np.full(n_ch[t] * CHUNK, -1.0, np.float32)
                pad[:cnt] = (loc[s0:s0 + cnt] - t * PT).astype(np.float32)
                DL[c, t, :, :n_ch[t]] = pad.reshape(n_ch[t], CHUNK).T
        lo, hi = c * NPER, min((c + 1) * NPER, N)
        NFT[c, :, :hi - lo] = nf[lo:hi].T
        ipad = np.ones(NT * PT, np.float32)
        ipad[:hi - lo] = invdeg[lo:hi]
        DL[c, :, :, MAXJ] = ipad.reshape(NT, PT)

    return dict(N=N, E=E, NPER=NPER, NT=NT, E_pad=E_pad, MAXJ=MAXJ,
                n_ch=n_ch, offs=offs, starts=starts, K=K, perm=perm,
                A1=A1, A2=A2, DL=DL, NFT=NFT)


# --------------------------------------------------------------------------
# device program
# --------------------------------------------------------------------------

def _build(meta, has_be2, has_bn2):
    import concourse.bass as bass
    import concourse.tile as tile
    from concourse import bacc, mybir

    f32 = mybir.dt.float32
    dt1 = mybir.dt.bfloat16 if STAGE1_BF16 else f32
    Alu = mybir.AluOpType
    Act = mybir.ActivationFunctionType

    NT, MAXJ, E_pad = meta["NT"], meta["MAXJ"], meta["E_pad"]
    n_ch = meta["n_ch"]

    nc = bacc.Bacc("TRN2", target_bir_lowering=False, debug=False,
                   enable_asserts=False, num_devices=NCORES)

    a1d = nc.dram_tensor("a1", [2 * D, E_pad], dt1, kind="ExternalInput").ap()
    a2d = nc.dram_tensor("a2", [D, E_pad], dt1, kind="ExternalInput").ap()
    dld = nc.dram_tensor("dl", [NT, PT, MAXJ + 1], f32, kind="ExternalInput").ap()
    nftd = nc.dram_tensor("nft", [D, NT * PT], f32, kind="ExternalInput").ap()
    w1ed = nc.dram_tensor("w1e", [2 * D, H], dt1, kind="ExternalInput").ap()
    w1bed = nc.dram_tensor("w1be", [D, H], dt1, kind="ExternalInput").ap()
    w1ad = nc.dram_tensor("w1a", [2 * D, H], dt1, kind="ExternalInput").ap()
    w1bad = nc.dram_tensor("w1ba", [D, H], dt1, kind="ExternalInput").ap()
    w2ed = nc.dram_tensor("w2e", [H, D], f32, kind="ExternalInput").ap()
    w2ad = nc.dram_tensor("w2a", [H, 1], f32, kind="ExternalInput").ap()
    wn1d = nc.dram_tensor("wn1", [2 * D, H], f32, kind="ExternalInput").ap()
    wn2d = nc.dram_tensor("wn2", [H, D], f32, kind="ExternalInput").ap()
    be1d = nc.dram_tensor("be1", [H], f32, kind="ExternalInput").ap()
    ba1d = nc.dram_tensor("ba1", [H], f32, kind="ExternalInput").ap()
    bn1d = nc.dram_tensor("bn1", [H], f32, kind="ExternalInput").ap()
    ba2d = nc.dram_tensor("ba2r", [PT], f32, kind="ExternalInput").ap()
    be2d = nc.dram_tensor("be2r", [PT, D], f32, kind="ExternalInput").ap()
    bn2d = nc.dram_tensor("bn2r", [PT, D], f32, kind="ExternalInput").ap()
    iotad = nc.dram_tensor("iota", [PT, PT], f32, kind="ExternalInput").ap()
    idnd = nc.dram_tensor("idn", [PT, PT], f32, kind="ExternalInput").ap()

    uefd = nc.dram_tensor("uef_out", [E_pad, D], f32, kind="ExternalOutput").ap()
    unfd = nc.dram_tensor("unf_out", [NT * PT, D], f32, kind="ExternalOutput").ap()

    # persistent uef slabs (double buffered across node tiles); col 64 of each
    # 65-wide chunk strip holds the constant 1.0 used to segment-sum p.
    slabs = [nc.alloc_sbuf_tensor(f"slab{i}", [PT, MAXJ * 65], f32).ap()
             for i in range(2)]

    with tile.TileContext(nc) as tc:
        with tc.tile_pool(name="const", bufs=1) as cpool, \
             tc.tile_pool(name="a1p", bufs=2) as a1pool, \
             tc.tile_pool(name="a2p", bufs=2) as a2pool, \
             tc.tile_pool(name="dlp", bufs=2) as dlpool, \
             tc.tile_pool(name="hsb", bufs=4) as hpool, \
             tc.tile_pool(name="small", bufs=4) as spool, \
             tc.tile_pool(name="msk", bufs=4) as mpool, \
             tc.tile_pool(name="node", bufs=2) as npool, \
             tc.tile_pool(name="hps", bufs=2, space="PSUM") as hps, \
             tc.tile_pool(name="ulps", bufs=2, space="PSUM") as ulps, \
             tc.tile_pool(name="outps", bufs=1, space="PSUM") as outps, \
             tc.tile_pool(name="nps", bufs=2, space="PSUM") as nps:

            w1e = cpool.tile_from(w1ed)
            w1be = cpool.tile_from(w1bed)
            w1a = cpool.tile_from(w1ad)
            w1ba = cpool.tile_from(w1bad)
            w2e = cpool.tile_from(w2ed)
            w2a = cpool.tile_from(w2ad)
            wn1 = cpool.tile_from(wn1d)
            wn2 = cpool.tile_from(wn2d)
            be1 = cpool.tile_from(be1d[:, None])
            ba1 = cpool.tile_from(ba1d[:, None])
            bn1 = cpool.tile_from(bn1d[:, None])
            ba2 = cpool.tile_from(ba2d[:, None])
            iota = cpool.tile_from(iotad)
            idn = cpool.tile_from(idnd)
            be2 = cpool.tile_from(be2d) if has_be2 else None
            bn2 = cpool.tile_from(bn2d) if has_bn2 else None

            for s in slabs:
                ones = s.rearrange("p (j c) -> p j c", c=65)[:, :, 64:65]
                nc.vector.memset(ones, 1.0)

            for t in range(NT):
                nj = int(n_ch[t])
                o = int(meta["offs"][t])
                ncols = nj * CHUNK
                slab = slabs[t % 2]

                a1 = a1pool.tile([2 * D, MAXJ * CHUNK], dt1, tag="a1")
                a2 = a2pool.tile([D, MAXJ * CHUNK], dt1, tag="a2")
                dl = dlpool.tile([PT, MAXJ + 1], f32, tag="dl")
                nc.sync.dma_start(out=a1[:, :ncols], in_=a1d[:, o:o + ncols])
                nc.sync.dma_start(out=a2[:, :ncols], in_=a2d[:, o:o + ncols])
                nc.sync.dma_start(out=dl[:], in_=dld[t])

                outp = outps.tile([PT, 65], f32, tag="outp")

                for g in range(-(-nj // GB)):
                    w = min(GB, nj - g * GB) * CHUNK
                    c0 = g * GB * CHUNK
                    he = hps.tile([H, GB * CHUNK], f32, tag="hps")
                    ha = hps.tile([H, GB * CHUNK], f32, tag="hps")
                    nc.tensor.matmul(out=he[:, :w], lhsT=w1e[:],
                                     rhs=a1[:, c0:c0 + w], start=True, stop=False)
                    nc.tensor.matmul(out=he[:, :w], lhsT=w1be[:],
                                     rhs=a2[:, c0:c0 + w], start=False, stop=True)
                    nc.tensor.matmul(out=ha[:, :w], lhsT=w1a[:],
                                     rhs=a1[:, c0:c0 + w], start=True, stop=False)
                    nc.tensor.matmul(out=ha[:, :w], lhsT=w1ba[:],
                                     rhs=a2[:, c0:c0 + w], start=False, stop=True)
                    hesb = hpool.tile([H, GB * CHUNK], f32, tag="hesb")
                    hasb = hpool.tile([H, GB * CHUNK], f32, tag="hasb")
                    nc.scalar.activation(out=hesb[:, :w], in_=he[:, :w],
                                         func=Act.Relu, bias=be1[:])
                    nc.vector.tensor_scalar(out=hasb[:, :w], in0=ha[:, :w],
                                            scalar1=ba1[:], scalar2=0.0,
                                            op0=Alu.add, op1=Alu.max)

                    for j4 in range(w // CHUNK):
                        j = g * GB + j4
                        cc = j4 * CHUNK
                        ps = ulps.tile([PT, 65], f32, tag="ulps")
                        nc.tensor.matmul(out=ps[:, 0:D],
                                         lhsT=hesb[:, cc:cc + CHUNK],
                                         rhs=w2e[:], start=True, stop=True)
                        nc.tensor.matmul(out=ps[:, D:D + 1],
                                         lhsT=hasb[:, cc:cc + CHUNK],
                                         rhs=w2a[:], start=True, stop=True)
                        p = spool.tile([PT, 1], f32, tag="p")
                        nc.scalar.activation(out=p[:], in_=ps[:, D:D + 1],
                                             func=Act.Exp, bias=ba2[:])
                        dst_sl = slab[:, j * 65:j * 65 + D]
                        if be2 is not None:
                            nc.vector.tensor_tensor(out=dst_sl, in0=ps[:, 0:D],
                                                    in1=be2[:], op=Alu.add)
                        elif j % 2 == 0:
                            nc.vector.tensor_copy(out=dst_sl, in_=ps[:, 0:D])
                        else:
                            nc.scalar.copy(out=dst_sl, in_=ps[:, 0:D])
                        msk = mpool.tile([PT, PT], f32, tag="msk")
                        nc.vector.tensor_scalar(out=msk[:], in0=iota[:],
                                                scalar1=dl[:, j:j + 1],
                                                scalar2=p[:, 0:1],
                                                op0=Alu.is_equal, op1=Alu.mult)
                        nc.tensor.matmul(out=outp[:], lhsT=msk[:],
                                         rhs=slab[:, j * 65:j * 65 + 65],
                                         start=(j == 0), stop=(j == nj - 1),
                                         skip_group_check=True)

                uef_view = uefd[o:o + ncols, :].rearrange(
                    "(j p) f -> p j f", p=PT)
                slab_view = slab.rearrange("p (j c) -> p j c", c=65)[:, :nj, 0:D]
                nc.sync.dma_start(out=uef_view, in_=slab_view)

                # ---- node phase ----
                r = spool.tile([PT, 1], f32, tag="r")
                nc.vector.tensor_scalar(out=r[:], in0=outp[:, D:D + 1],
                                        scalar1=1e-30, scalar2=None, op0=Alu.max)
                nc.vector.reciprocal(out=r[:], in_=r[:])
                nc.vector.tensor_scalar(out=r[:], in0=r[:],
                                        scalar1=dl[:, MAXJ:MAXJ + 1],
                                        scalar2=None, op0=Alu.mult)
                agg = spool.tile([PT, D], f32, tag="agg")
                nc.vector.tensor_scalar(out=agg[:], in0=outp[:, 0:D],
                                        scalar1=r[:], scalar2=None, op0=Alu.mult)
                tp = nps.tile([D, PT], f32, tag="nps")
                nc.tensor.transpose(tp[:], agg[:], idn[:])
                nin = npool.tile([2 * D, PT], f32, tag="nin")
                nc.scalar.copy(out=nin[0:D, :], in_=tp[:])
                nc.sync.dma_start(out=nin[D:2 * D, :],
                                  in_=nftd[:, t * PT:(t + 1) * PT])
                hn_ps = nps.tile([H, PT], f32, tag="nps")
                nc.tensor.matmul(out=hn_ps[:], lhsT=wn1[:], rhs=nin[:],
                                 start=True, stop=True)
                hn = npool.tile([H, PT], f32, tag="hn")
                nc.scalar.activation(out=hn[:], in_=hn_ps[:],
                                     func=Act.Relu, bias=bn1[:])
                unf_ps = nps.tile([PT, D], f32, tag="nps")
                nc.tensor.matmul(out=unf_ps[:], lhsT=hn[:], rhs=wn2[:],
                                 start=True, stop=True)
                unf_sb = npool.tile([PT, D], f32, tag="unfsb")
                if bn2 is not None:
                    nc.vector.tensor_tensor(out=unf_sb[:], in0=unf_ps[:],
                                            in1=bn2[:], op=Alu.add)
                else:
                    nc.vector.tensor_copy(out=unf_sb[:], in_=unf_ps[:])
                nc.sync.dma_start(out=unfd[t * PT:(t + 1) * PT, :], in_=unf_sb[:])

    nc.compile()
    return nc


# --------------------------------------------------------------------------
# entry point
# --------------------------------------------------------------------------

def kernel(nf, ef, We1, be1, We2, be2, Wa1, ba1, Wa2, ba2,
           Wn1, bn1, Wn2, bn2, src, dst):
    import ml_dtypes
    from concourse.bass_utils import run_bass_kernel_spmd

    nf = np.ascontiguousarray(np.asarray(nf, np.float32))
    ef = np.ascontiguousarray(np.asarray(ef, np.float32))
    meta = _prep(nf, ef, np.asarray(src), np.asarray(dst))

    has_be2 = bool(np.any(np.asarray(be2)))
    has_bn2 = bool(np.any(np.asarray(bn2)))

    key = (meta["E_pad"], meta["MAXJ"], tuple(meta["n_ch"].tolist()),
           has_be2, has_bn2, STAGE1_BF16)
    if key not in _PROG_CACHE:
        _PROG_CACHE[key] = _build(meta, has_be2, has_bn2)
    nc = _PROG_CACHE[key]

    cast1 = (lambda a: np.asarray(a, np.float32).astype(ml_dtypes.bfloat16)) \
        if STAGE1_BF16 else (lambda a: np.asarray(a, np.float32))
    f32 = lambda a: np.ascontiguousarray(np.asarray(a, np.float32))

    shared = {
        "w1e": cast1(We1[:2 * D]), "w1be": cast1(We1[2 * D:]),
        "w1a": cast1(Wa1[:2 * D]), "w1ba": cast1(Wa1[2 * D:]),
        "w2e": f32(We2), "w2a": f32(Wa2),
        "wn1": f32(Wn1), "wn2": f32(Wn2),
        "be1": f32(be1), "ba1": f32(ba1), "bn1": f32(bn1),
        "ba2r": np.full(PT, np.float32(np.asarray(ba2).reshape(-1)[0])),
        "be2r": np.broadcast_to(f32(be2), (PT, D)).copy(),
        "bn2r": np.broadcast_to(f32(bn2), (PT, D)).copy(),
        "iota": np.broadcast_to(np.arange(PT, dtype=np.float32), (PT, PT)).copy(),
        "idn": np.eye(PT, dtype=np.float32),
    }
    in_maps = []
    for c in range(NCORES):
        m = dict(shared)
        m["a1"] = cast1(meta["A1"][c])
        m["a2"] = cast1(meta["A2"][c])
        m["dl"] = meta["DL"][c]
        m["nft"] = meta["NFT"][c]
        in_maps.append(m)

    res = run_bass_kernel_spmd(nc, in_maps, core_ids=list(range(NCORES)))
    global _LAST_RUN
    _LAST_RUN = res

    N, E, NPER, NT = meta["N"], meta["E"], meta["NPER"], meta["NT"]
    n_ch, offs, starts, K, perm = (meta["n_ch"], meta["offs"],
                                   meta["starts"], meta["K"], meta["perm"])
    unf = np.empty((N, D), np.float32)
    uef = np.empty((E, D), np.float32)
    for c in range(NCORES):
        lo, hi = c * NPER, min((c + 1) * NPER, N)
        unf[lo:hi] = res.results[c]["unf_out"][:hi - lo]
        uo = res.results[c]["uef_out"]
        for t in range(NT):
            g = c * NT + t
            s0, cnt = starts[g], K[c, t]
            if cnt:
                o = offs[t]
                uef[perm[s0:s0 + cnt]] = uo[o:o + cnt]
    return unf, uef


# revision 13
# speedup vs baseline: 2.2779x; 2.2779x over previous
"""Trainium2 Bass kernel for an attention MPNN layer (edge MLP + segment
softmax + scatter-mean + node MLP), distributed over 8 NeuronCores.

Strategy: host sorts edges by destination node and partitions BOTH the nodes
and their incoming edges across the 8 cores (node range [c*N/8,(c+1)*N/8) and
every edge pointing into it live on core c). Each core is then fully
independent -- no collectives. Within a core, nodes are processed in tiles of
128; a tile's incoming edges stream through the edge MLPs in chunks of 128,
and the segment softmax/mean reduction is realised as a mask matmul
(mask[e, n] = p_e * [dst_e == n]) accumulated in PSUM across the tile's
chunks. All irregularity (gather of nf[src]/nf[dst], sort, ragged segment
boundaries) is resolved on the host, so the device only executes dense
DMAs and matmuls.
"""

import math

import numpy as np

D = 64
H = 128
NCORES = 8
PT = 128  # nodes per tile (partition dim)
CHUNK = 128  # edges per sub-chunk
GB = 4  # sub-chunks per h-stage group (moving-dim 512)

# dtype config: stage-1 stream (edge features + layer-1 weights), layer-2
# (h activations + We2/Wa2), and the segment mask matmul. fp32 PSUM
# accumulation and fp32 softmax/normalization/node-MLP throughout.
STAGE1_BF16 = True
L2_BF16 = True
SEG_BF16 = True

_PROG_CACHE = {}


# --------------------------------------------------------------------------
# host-side preprocessing
# --------------------------------------------------------------------------

def _prep(nf, ef, src, dst):
    N, E = nf.shape[0], ef.shape[0]
    NPER = -(-N // NCORES)
    NT = -(-NPER // PT)

    src = np.ascontiguousarray(src).astype(np.int64, copy=False)
    dst = np.ascontiguousarray(dst).astype(np.int64, copy=False)

    perm = np.argsort(dst, kind="stable")
    dsts = dst[perm]
    srcs = src[perm]
    core_e = dsts // NPER
    loc = dsts - core_e * NPER
    tloc = loc // PT
    gid = core_e * NT + tloc  # nondecreasing
    starts = np.searchsorted(gid, np.arange(NCORES * NT + 1), side="left")
    K = np.diff(starts).reshape(NCORES, NT)

    n_ch = np.maximum(1, -(-K.max(axis=0) // CHUNK)).astype(np.int64)  # [NT]
    offs = np.zeros(NT + 1, np.int64)
    offs[1:] = np.cumsum(n_ch) * CHUNK
    E_pad = int(offs[-1])
    MAXJ = int(n_ch.max())

    efs = ef[perm]
    nfss = nf[srcs]
    nfds = nf[dsts]
    deg = np.bincount(dst, minlength=N).astype(np.float64)
    invdeg = (1.0 / np.maximum(deg, 1.0)).astype(np.float32)

    A1 = np.zeros((NCORES, 2 * D, E_pad), np.float32)
    A2 = np.zeros((NCORES, D, E_pad), np.float32)
    DL = np.full((NCORES, NT, PT, MAXJ + 1), -1.0, np.float32)
    NFT = np.zeros((NCORES, D, NT * PT), np.float32)

    for c in range(NCORES):
        for t in range(NT):
            g = c * NT + t
            s0, cnt = starts[g], K[c, t]
            o = offs[t]
            if cnt:
                A1[c, :D, o:o + cnt] = efs[s0:s0 + cnt].T
                A1[c, D:, o:o + cnt] = nfss[s0:s0 + cnt].T
                A2[c, :, o:o + cnt] = nfds[s0:s0 + cnt].T
                pad = np.full(n_ch[t] * CHUNK, -1.0, np.float32)
                pad[:cnt] = (loc[s0:s0 + cnt] - t * PT).astype(np.float32)
                DL[c, t, :, :n_ch[t]] = pad.reshape(n_ch[t], CHUNK).T
        lo, hi = c * NPER, min((c + 1) * NPER, N)
        NFT[c, :, :hi - lo] = nf[lo:hi].T
        ipad = np.ones(NT * PT, np.float32)
        ipad[:hi - lo] = invdeg[lo:hi]
        DL[c, :, :, MAXJ] = ipad.reshape(NT, PT)

    return dict(N=N, E=E, NPER=NPER, NT=NT, E_pad=E_pad, MAXJ=MAXJ,
                n_ch=n_ch, offs=offs, starts=starts, K=K, perm=perm,
                A1=A1, A2=A2, DL=DL, NFT=NFT)


# --------------------------------------------------------------------------
# device program
# --------------------------------------------------------------------------

def _build(meta, has_be2, has_bn2):
    import concourse.bass as bass
    import concourse.tile as tile
    from concourse import bacc, mybir

    f32 = mybir.dt.float32
    bf16 = mybir.dt.bfloat16
    dt1 = bf16 if STAGE1_BF16 else f32
    dt2 = bf16 if L2_BF16 else f32
    dts = bf16 if SEG_BF16 else f32
    Alu = mybir.AluOpType
    Act = mybir.ActivationFunctionType

    NT, MAXJ, E_pad = meta["NT"], meta["MAXJ"], meta["E_pad"]
    n_ch = meta["n_ch"]

    nc = bacc.Bacc("TRN2", target_bir_lowering=False, debug=False,
                   enable_asserts=False, num_devices=NCORES)

    a1d = nc.dram_tensor("a1", [2 * D, E_pad], dt1, kind="ExternalInput").ap()
    a2d = nc.dram_tensor("a2", [D, E_pad], dt1, kind="ExternalInput").ap()
    dld = nc.dram_tensor("dl", [NT, PT, MAXJ + 1], f32, kind="ExternalInput").ap()
    nftd = nc.dram_tensor("nft", [D, NT * PT], f32, kind="ExternalInput").ap()
    w1ed = nc.dram_tensor("w1e", [2 * D, H], dt1, kind="ExternalInput").ap()
    w1bed = nc.dram_tensor("w1be", [D, H], dt1, kind="ExternalInput").ap()
    w1ad = nc.dram_tensor("w1a", [2 * D, H], dt1, kind="ExternalInput").ap()
    w1bad = nc.dram_tensor("w1ba", [D, H], dt1, kind="ExternalInput").ap()
    w2ed = nc.dram_tensor("w2e", [H, D], dt2, kind="ExternalInput").ap()
    w2ad = nc.dram_tensor("w2a", [H, 1], dt2, kind="ExternalInput").ap()
    wn1d = nc.dram_tensor("wn1", [2 * D, H], f32, kind="ExternalInput").ap()
    wn2d = nc.dram_tensor("wn2", [H, D], f32, kind="ExternalInput").ap()
    be1d = nc.dram_tensor("be1", [H], f32, kind="ExternalInput").ap()
    ba1d = nc.dram_tensor("ba1", [H], f32, kind="ExternalInput").ap()
    bn1d = nc.dram_tensor("bn1", [H], f32, kind="ExternalInput").ap()
    ba2d = nc.dram_tensor("ba2r", [PT], f32, kind="ExternalInput").ap()
    be2d = nc.dram_tensor("be2r", [PT, D], f32, kind="ExternalInput").ap()
    bn2d = nc.dram_tensor("bn2r", [PT, D], f32, kind="ExternalInput").ap()
    iotad = nc.dram_tensor("iota", [PT, PT], f32, kind="ExternalInput").ap()
    idnd = nc.dram_tensor("idn", [PT, PT], f32, kind="ExternalInput").ap()

    uefd = nc.dram_tensor("uef_out", [E_pad, D], f32, kind="ExternalOutput").ap()
    unfd = nc.dram_tensor("unf_out", [NT * PT, D], f32, kind="ExternalOutput").ap()

    # persistent uef slabs (double buffered across node tiles); col 64 of each
    # 65-wide chunk strip holds the constant 1.0 used to segment-sum p. The
    # fp32 slabs feed the uef output DMA; when SEG_BF16 a parallel bf16 pair
    # feeds the segment matmul.
    slabs = [nc.alloc_sbuf_tensor(f"slab{i}", [PT, MAXJ * 65], f32).ap()
             for i in range(2)]
    if SEG_BF16:
        bslabs = [nc.alloc_sbuf_tensor(f"bslab{i}", [PT, MAXJ * 65], dts).ap()
                  for i in range(2)]
    else:
        bslabs = slabs

    with tile.TileContext(nc) as tc:
        with tc.tile_pool(name="const", bufs=1) as cpool, \
             tc.tile_pool(name="a1p", bufs=2) as a1pool, \
             tc.tile_pool(name="a2p", bufs=2) as a2pool, \
             tc.tile_pool(name="dlp", bufs=2) as dlpool, \
             tc.tile_pool(name="hsb", bufs=4) as hpool, \
             tc.tile_pool(name="small", bufs=4) as spool, \
             tc.tile_pool(name="msk", bufs=4) as mpool, \
             tc.tile_pool(name="node", bufs=2) as npool, \
             tc.tile_pool(name="hps", bufs=2, space="PSUM") as hps, \
             tc.tile_pool(name="ulps", bufs=2, space="PSUM") as ulps, \
             tc.tile_pool(name="outps", bufs=1, space="PSUM") as outps, \
             tc.tile_pool(name="nps", bufs=2, space="PSUM") as nps:

            w1e = cpool.tile_from(w1ed)
            w1be = cpool.tile_from(w1bed)
            w1a = cpool.tile_from(w1ad)
            w1ba = cpool.tile_from(w1bad)
            w2e = cpool.tile_from(w2ed)
            w2a = cpool.tile_from(w2ad)
            wn1 = cpool.tile_from(wn1d)
            wn2 = cpool.tile_from(wn2d)
            be1 = cpool.tile_from(be1d[:, None])
            ba1 = cpool.tile_from(ba1d[:, None])
            bn1 = cpool.tile_from(bn1d[:, None])
            ba2 = cpool.tile_from(ba2d[:, None])
            iota = cpool.tile_from(iotad)
            idn = cpool.tile_from(idnd)
            be2 = cpool.tile_from(be2d) if has_be2 else None
            bn2 = cpool.tile_from(bn2d) if has_bn2 else None

            for s in bslabs:
                ones = s.rearrange("p (j c) -> p j c", c=65)[:, :, 64:65]
                nc.vector.memset(ones, 1.0)

            for t in range(NT):
                nj = int(n_ch[t])
                o = int(meta["offs"][t])
                ncols = nj * CHUNK
                slab = slabs[t % 2]
                bslab = bslabs[t % 2]

                a1 = a1pool.tile([2 * D, MAXJ * CHUNK], dt1, tag="a1")
                a2 = a2pool.tile([D, MAXJ * CHUNK], dt1, tag="a2")
                dl = dlpool.tile([PT, MAXJ + 1], f32, tag="dl")
                nc.sync.dma_start(out=a1[:, :ncols], in_=a1d[:, o:o + ncols])
                nc.sync.dma_start(out=a2[:, :ncols], in_=a2d[:, o:o + ncols])
                nc.sync.dma_start(out=dl[:], in_=dld[t])

                outp = outps.tile([PT, 65], f32, tag="outp")

                for g in range(-(-nj // GB)):
                    w = min(GB, nj - g * GB) * CHUNK
                    c0 = g * GB * CHUNK
                    he = hps.tile([H, GB * CHUNK], f32, tag="hps")
                    ha = hps.tile([H, GB * CHUNK], f32, tag="hps")
                    nc.tensor.matmul(out=he[:, :w], lhsT=w1e[:],
                                     rhs=a1[:, c0:c0 + w], start=True, stop=False)
                    nc.tensor.matmul(out=he[:, :w], lhsT=w1be[:],
                                     rhs=a2[:, c0:c0 + w], start=False, stop=True)
                    nc.tensor.matmul(out=ha[:, :w], lhsT=w1a[:],
                                     rhs=a1[:, c0:c0 + w], start=True, stop=False)
                    nc.tensor.matmul(out=ha[:, :w], lhsT=w1ba[:],
                                     rhs=a2[:, c0:c0 + w], start=False, stop=True)
                    hesb = hpool.tile([H, GB * CHUNK], dt2, tag="hesb")
                    hasb = hpool.tile([H, GB * CHUNK], dt2, tag="hasb")
                    nc.scalar.activation(out=hesb[:, :w], in_=he[:, :w],
                                         func=Act.Relu, bias=be1[:])
                    nc.vector.tensor_scalar(out=hasb[:, :w], in0=ha[:, :w],
                                            scalar1=ba1[:], scalar2=0.0,
                                            op0=Alu.add, op1=Alu.max)

                    for j4 in range(w // CHUNK):
                        j = g * GB + j4
                        cc = j4 * CHUNK
                        ps = ulps.tile([PT, 65], f32, tag="ulps")
                        nc.tensor.matmul(out=ps[:, 0:D],
                                         lhsT=hesb[:, cc:cc + CHUNK],
                                         rhs=w2e[:], start=True, stop=True)
                        nc.tensor.matmul(out=ps[:, D:D + 1],
                                         lhsT=hasb[:, cc:cc + CHUNK],
                                         rhs=w2a[:], start=True, stop=True)
                        p = spool.tile([PT, 1], f32, tag="p")
                        nc.scalar.activation(out=p[:], in_=ps[:, D:D + 1],
                                             func=Act.Exp, bias=ba2[:])
                        def _copy(dst, src, on_vector):
                            if on_vector:
                                nc.vector.tensor_copy(out=dst, in_=src)
                            else:
                                nc.scalar.copy(out=dst, in_=src)

                        dst_sl = slab[:, j * 65:j * 65 + D]
                        bdst_sl = bslab[:, j * 65:j * 65 + D]
                        if be2 is not None:
                            nc.vector.tensor_tensor(out=dst_sl, in0=ps[:, 0:D],
                                                    in1=be2[:], op=Alu.add)
                            if SEG_BF16:
                                nc.scalar.copy(out=bdst_sl, in_=dst_sl)
                        else:
                            _copy(dst_sl, ps[:, 0:D], j % 2 == 0)
                            if SEG_BF16:
                                _copy(bdst_sl, ps[:, 0:D], j % 2 == 1)
                        msk = mpool.tile([PT, PT], dts, tag="msk")
                        nc.vector.tensor_scalar(out=msk[:], in0=iota[:],
                                                scalar1=dl[:, j:j + 1],
                                                scalar2=p[:, 0:1],
                                                op0=Alu.is_equal, op1=Alu.mult)
                        nc.tensor.matmul(out=outp[:], lhsT=msk[:],
                                         rhs=bslab[:, j * 65:j * 65 + 65],
                                         start=(j == 0), stop=(j == nj - 1),
                                         skip_group_check=True)

                uef_view = uefd[o:o + ncols, :].rearrange(
                    "(j p) f -> p j f", p=PT)
                slab_view = slab.rearrange("p (j c) -> p j c", c=65)[:, :nj, 0:D]
                nc.sync.dma_start(out=uef_view, in_=slab_view)

                # ---- node phase ----
                r = spool.tile([PT, 1], f32, tag="r")
                nc.vector.tensor_scalar(out=r[:], in0=outp[:, D:D + 1],
                                        scalar1=1e-30, scalar2=None, op0=Alu.max)
                nc.vector.reciprocal(out=r[:], in_=r[:])
                nc.vector.tensor_scalar(out=r[:], in0=r[:],
                                        scalar1=dl[:, MAXJ:MAXJ + 1],
                                        scalar2=None, op0=Alu.mult)
                agg = spool.tile([PT, D], f32, tag="agg")
                nc.vector.tensor_scalar(out=agg[:], in0=outp[:, 0:D],
                                        scalar1=r[:], scalar2=None, op0=Alu.mult)
                tp = nps.tile([D, PT], f32, tag="nps")
                nc.tensor.transpose(tp[:], agg[:], idn[:])
                nin = npool.tile([2 * D, PT], f32, tag="nin")
                nc.scalar.copy(out=nin[0:D, :], in_=tp[:])
                nc.sync.dma_start(out=nin[D:2 * D, :],
                                  in_=nftd[:, t * PT:(t + 1) * PT])
                hn_ps = nps.tile([H, PT], f32, tag="nps")
                nc.tensor.matmul(out=hn_ps[:], lhsT=wn1[:], rhs=nin[:],
                                 start=True, stop=True)
                hn = npool.tile([H, PT], f32, tag="hn")
                nc.scalar.activation(out=hn[:], in_=hn_ps[:],
                                     func=Act.Relu, bias=bn1[:])
                unf_ps = nps.tile([PT, D], f32, tag="nps")
                nc.tensor.matmul(out=unf_ps[:], lhsT=hn[:], rhs=wn2[:],
                                 start=True, stop=True)
                unf_sb = npool.tile([PT, D], f32, tag="unfsb")
                if bn2 is not None:
                    nc.vector.tensor_tensor(out=unf_sb[:], in0=unf_ps[:],
                                            in1=bn2[:], op=Alu.add)
                else:
                    nc.vector.tensor_copy(out=unf_sb[:], in_=unf_ps[:])
                nc.sync.dma_start(out=unfd[t * PT:(t + 1) * PT, :], in_=unf_sb[:])

    nc.compile()
    return nc


# --------------------------------------------------------------------------
# entry point
# --------------------------------------------------------------------------

def kernel(nf, ef, We1, be1, We2, be2, Wa1, ba1, Wa2, ba2,
           Wn1, bn1, Wn2, bn2, src, dst):
    import ml_dtypes
    from concourse.bass_utils import run_bass_kernel_spmd

    nf = np.ascontiguousarray(np.asarray(nf, np.float32))
    ef = np.ascontiguousarray(np.asarray(ef, np.float32))
    meta = _prep(nf, ef, np.asarray(src), np.asarray(dst))

    has_be2 = bool(np.any(np.asarray(be2)))
    has_bn2 = bool(np.any(np.asarray(bn2)))

    key = (meta["E_pad"], meta["MAXJ"], tuple(meta["n_ch"].tolist()),
           has_be2, has_bn2, STAGE1_BF16, L2_BF16, SEG_BF16)
    if key not in _PROG_CACHE:
        _PROG_CACHE[key] = _build(meta, has_be2, has_bn2)
    nc = _PROG_CACHE[key]

    bfc = lambda a: np.ascontiguousarray(np.asarray(a, np.float32)).astype(
        ml_dtypes.bfloat16)
    f32 = lambda a: np.ascontiguousarray(np.asarray(a, np.float32))
    cast1 = bfc if STAGE1_BF16 else f32
    cast2 = bfc if L2_BF16 else f32

    shared = {
        "w1e": cast1(We1[:2 * D]), "w1be": cast1(We1[2 * D:]),
        "w1a": cast1(Wa1[:2 * D]), "w1ba": cast1(Wa1[2 * D:]),
        "w2e": cast2(We2), "w2a": cast2(Wa2),
        "wn1": f32(Wn1), "wn2": f32(Wn2),
        "be1": f32(be1), "ba1": f32(ba1), "bn1": f32(bn1),
        "ba2r": np.full(PT, np.float32(np.asarray(ba2).reshape(-1)[0])),
        "be2r": np.broadcast_to(f32(be2), (PT, D)).copy(),
        "bn2r": np.broadcast_to(f32(bn2), (PT, D)).copy(),
        "iota": np.broadcast_to(np.arange(PT, dtype=np.float32), (PT, PT)).copy(),
        "idn": np.eye(PT, dtype=np.float32),
    }
    in_maps = []
    for c in range(NCORES):
        m = dict(shared)
        m["a1"] = cast1(meta["A1"][c])
        m["a2"] = cast1(meta["A2"][c])
        m["dl"] = meta["DL"][c]
        m["nft"] = meta["NFT"][c]
        in_maps.append(m)

    res = run_bass_kernel_spmd(nc, in_maps, core_ids=list(range(NCORES)))
    global _LAST_RUN
    _LAST_RUN = res

    N, E, NPER, NT = meta["N"], meta["E"], meta["NPER"], meta["NT"]
    n_ch, offs, starts, K, perm = (meta["n_ch"], meta["offs"],
                                   meta["starts"], meta["K"], meta["perm"])
    unf = np.empty((N, D), np.float32)
    uef = np.empty((E, D), np.float32)
    for c in range(NCORES):
        lo, hi = c * NPER, min((c + 1) * NPER, N)
        unf[lo:hi] = res.results[c]["unf_out"][:hi - lo]
        uo = res.results[c]["uef_out"]
        for t in range(NT):
            g = c * NT + t
            s0, cnt = starts[g], K[c, t]
            if cnt:
                o = offs[t]
                uef[perm[s0:s0 + cnt]] = uo[o:o + cnt]
    return unf, uef


# revision 27
# speedup vs baseline: 3.3283x; 1.4611x over previous
"""Trainium2 Bass kernel for an attention MPNN layer (edge MLP + segment
softmax + scatter-mean + node MLP), distributed over 8 NeuronCores.

Strategy: host sorts edges by destination node and partitions BOTH the nodes
and their incoming edges across the 8 cores (node range [c*N/8,(c+1)*N/8) and
every edge pointing into it live on core c). Each core is then fully
independent -- no collectives. Within a core, nodes are processed in tiles of
128; a tile's incoming edges stream through the edge MLPs in chunks of 128,
and the segment softmax/mean reduction is realised as a mask matmul
(mask[e, n] = p_e * [dst_e == n]) accumulated in PSUM across the tile's
chunks. All irregularity (gather of nf[src]/nf[dst], sort, ragged segment
boundaries) is resolved on the host, so the device only executes dense
DMAs and matmuls.
"""

import math

import numpy as np

D = 64
H = 128
NCORES = 8
PT = 128  # nodes per tile (partition dim)
CHUNK = 128  # edges per sub-chunk
GB = 4  # sub-chunks per h-stage group (moving-dim 512)

# dtype config: stage-1 stream (edge features + layer-1 weights), layer-2
# (h activations + We2/Wa2), and the segment mask matmul. fp32 PSUM
# accumulation and fp32 softmax/normalization/node-MLP throughout.
STAGE1_BF16 = True
L2_BF16 = True
SEG_BF16 = True
NODE_BF16 = True

_PROG_CACHE = {}


# --------------------------------------------------------------------------
# host-side preprocessing
# --------------------------------------------------------------------------

def _prep(nf, ef, src, dst):
    N, E = nf.shape[0], ef.shape[0]
    NPER = -(-N // NCORES)
    NT = -(-NPER // PT)

    src = np.ascontiguousarray(src).astype(np.int64, copy=False)
    dst = np.ascontiguousarray(dst).astype(np.int64, copy=False)

    perm = np.argsort(dst, kind="stable")
    dsts = dst[perm]
    srcs = src[perm]
    core_e = dsts // NPER
    loc = dsts - core_e * NPER
    tloc = loc // PT
    gid = core_e * NT + tloc  # nondecreasing
    starts = np.searchsorted(gid, np.arange(NCORES * NT + 1), side="left")
    K = np.diff(starts).reshape(NCORES, NT)

    n_ch = np.maximum(1, -(-K.max(axis=0) // CHUNK)).astype(np.int64)  # [NT]
    offs = np.zeros(NT + 1, np.int64)
    offs[1:] = np.cumsum(n_ch) * CHUNK
    E_pad = int(offs[-1])
    MAXJ = int(n_ch.max())

    efs = ef[perm]
    nfss = nf[srcs]
    nfds = nf[dsts]
    deg = np.bincount(dst, minlength=N).astype(np.float64)
    invdeg = (1.0 / np.maximum(deg, 1.0)).astype(np.float32)

    A1 = np.zeros((NCORES, 2 * D, E_pad), np.float32)
    A2 = np.zeros((NCORES, D, E_pad), np.float32)
    DL = np.full((NCORES, NT, PT, MAXJ + 1), -1.0, np.float32)
    NFT = np.zeros((NCORES, D, NT * PT), np.float32)

    for c in range(NCORES):
        for t in range(NT):
            g = c * NT + t
            s0, cnt = starts[g], K[c, t]
            o = offs[t]
            if cnt:
                A1[c, :D, o:o + cnt] = efs[s0:s0 + cnt].T
                A1[c, D:, o:o + cnt] = nfss[s0:s0 + cnt].T
                A2[c, :, o:o + cnt] = nfds[s0:s0 + cnt].T
                pad = np.full(n_ch[t] * CHUNK, -1.0, np.float32)
                pad[:cnt] = (loc[s0:s0 + cnt] - t * PT).astype(np.float32)
                DL[c, t, :, :n_ch[t]] = pad.reshape(n_ch[t], CHUNK).T
        lo, hi = c * NPER, min((c + 1) * NPER, N)
        NFT[c, :, :hi - lo] = nf[lo:hi].T
        ipad = np.ones(NT * PT, np.float32)
        ipad[:hi - lo] = invdeg[lo:hi]
        DL[c, :, :, MAXJ] = ipad.reshape(NT, PT)

    return dict(N=N, E=E, NPER=NPER, NT=NT, E_pad=E_pad, MAXJ=MAXJ,
                n_ch=n_ch, offs=offs, starts=starts, K=K, perm=perm,
                A1=A1, A2=A2, DL=DL, NFT=NFT)


# --------------------------------------------------------------------------
# device program
# --------------------------------------------------------------------------

def _build(meta, has_be2, has_bn2):
    import concourse.bass as bass
    import concourse.tile as tile
    from concourse import bacc, mybir

    f32 = mybir.dt.float32
    bf16 = mybir.dt.bfloat16
    dt1 = bf16 if STAGE1_BF16 else f32
    dt2 = bf16 if L2_BF16 else f32
    dts = bf16 if SEG_BF16 else f32
    dtn = bf16 if NODE_BF16 else f32
    Alu = mybir.AluOpType
    Act = mybir.ActivationFunctionType

    NT, MAXJ, E_pad = meta["NT"], meta["MAXJ"], meta["E_pad"]
    n_ch = meta["n_ch"]

    nc = bacc.Bacc("TRN2", target_bir_lowering=False, debug=False,
                   enable_asserts=False, num_devices=NCORES)

    a1d = nc.dram_tensor("a1", [2 * D, E_pad], dt1, kind="ExternalInput").ap()
    a2d = nc.dram_tensor("a2", [D, E_pad], dt1, kind="ExternalInput").ap()
    dld = nc.dram_tensor("dl", [NT, PT, MAXJ + 1], f32, kind="ExternalInput").ap()
    nftd = nc.dram_tensor("nft", [D, NT * PT], dtn, kind="ExternalInput").ap()
    w1ed = nc.dram_tensor("w1e", [2 * D, H], dt1, kind="ExternalInput").ap()
    w1bed = nc.dram_tensor("w1be", [D, H], dt1, kind="ExternalInput").ap()
    w1ad = nc.dram_tensor("w1a", [2 * D, H], dt1, kind="ExternalInput").ap()
    w1bad = nc.dram_tensor("w1ba", [D, H], dt1, kind="ExternalInput").ap()
    w2ed = nc.dram_tensor("w2e", [H, D], dt2, kind="ExternalInput").ap()
    w2ad = nc.dram_tensor("w2a", [H, 1], dt2, kind="ExternalInput").ap()
    wn1d = nc.dram_tensor("wn1", [2 * D, H], dtn, kind="ExternalInput").ap()
    wn2d = nc.dram_tensor("wn2", [H, D], dtn, kind="ExternalInput").ap()
    be1d = nc.dram_tensor("be1", [H], f32, kind="ExternalInput").ap()
    ba1d = nc.dram_tensor("ba1", [H], f32, kind="ExternalInput").ap()
    bn1d = nc.dram_tensor("bn1", [H], f32, kind="ExternalInput").ap()
    ba2d = nc.dram_tensor("ba2r", [PT], f32, kind="ExternalInput").ap()
    be2d = nc.dram_tensor("be2r", [PT, D], f32, kind="ExternalInput").ap()
    bn2d = nc.dram_tensor("bn2r", [PT, D], f32, kind="ExternalInput").ap()
    iotad = nc.dram_tensor("iota", [PT, PT], dts, kind="ExternalInput").ap()
    idnd = nc.dram_tensor("idn", [PT, PT], dtn, kind="ExternalInput").ap()

    uefd = nc.dram_tensor("uef_out", [E_pad, D], f32, kind="ExternalOutput").ap()
    unfd = nc.dram_tensor("unf_out", [NT * PT, D], f32, kind="ExternalOutput").ap()

    # persistent uef slabs (double buffered across node tiles); col 64 of each
    # 65-wide chunk strip holds the constant 1.0 used to segment-sum p. The
    # fp32 slabs feed the uef output DMA; when SEG_BF16 a parallel bf16 pair
    # feeds the segment matmul.
    slabs = [nc.alloc_sbuf_tensor(f"slab{i}", [PT, MAXJ * 65], f32).ap()
             for i in range(2)]
    if SEG_BF16:
        bslabs = [nc.alloc_sbuf_tensor(f"bslab{i}", [PT, MAXJ * 65], dts).ap()
                  for i in range(2)]
    else:
        bslabs = slabs

    with tile.TileContext(nc) as tc:
        with tc.tile_pool(name="const", bufs=1) as cpool, \
             tc.tile_pool(name="a1p", bufs=2) as a1pool, \
             tc.tile_pool(name="a2p", bufs=2) as a2pool, \
             tc.tile_pool(name="dlp", bufs=2) as dlpool, \
             tc.tile_pool(name="hsb", bufs=4) as hpool, \
             tc.tile_pool(name="small", bufs=4) as spool, \
             tc.tile_pool(name="msk", bufs=10) as mpool, \
             tc.tile_pool(name="node", bufs=2) as npool, \
             tc.tile_pool(name="hps", bufs=2, space="PSUM") as hps, \
             tc.tile_pool(name="ulps", bufs=2, space="PSUM") as ulps, \
             tc.tile_pool(name="outps", bufs=1, space="PSUM") as outps, \
             tc.tile_pool(name="nps", bufs=2, space="PSUM") as nps:

            w1e = cpool.tile_from(w1ed)
            w1be = cpool.tile_from(w1bed)
            w1a = cpool.tile_from(w1ad)
            w1ba = cpool.tile_from(w1bad)
            w2e = cpool.tile_from(w2ed)
            w2a = cpool.tile_from(w2ad)
            wn1 = cpool.tile_from(wn1d)
            wn2 = cpool.tile_from(wn2d)
            be1 = cpool.tile_from(be1d[:, None])
            ba1 = cpool.tile_from(ba1d[:, None])
            bn1 = cpool.tile_from(bn1d[:, None])
            ba2 = cpool.tile_from(ba2d[:, None])
            iota = cpool.tile_from(iotad)
            idn = cpool.tile_from(idnd)
            be2 = cpool.tile_from(be2d) if has_be2 else None
            bn2 = cpool.tile_from(bn2d) if has_bn2 else None

            for s in bslabs:
                ones = s.rearrange("p (j c) -> p j c", c=65)[:, :, 64:65]
                nc.vector.memset(ones, 1.0)

            for t in range(NT):
                nj = int(n_ch[t])
                o = int(meta["offs"][t])
                ncols = nj * CHUNK
                slab = slabs[t % 2]
                bslab = bslabs[t % 2]

                a1 = a1pool.tile([2 * D, MAXJ * CHUNK], dt1, tag="a1")
                a2 = a2pool.tile([D, MAXJ * CHUNK], dt1, tag="a2")
                dl = dlpool.tile([PT, MAXJ + 1], f32, tag="dl")
                nc.sync.dma_start(out=a1[:, :ncols], in_=a1d[:, o:o + ncols])
                nc.sync.dma_start(out=a2[:, :ncols], in_=a2d[:, o:o + ncols])
                nc.sync.dma_start(out=dl[:], in_=dld[t])

                outp = outps.tile([PT, 65], f32, tag="outp")

                def _copy(dst, src, on_vector):
                    if on_vector:
                        nc.vector.tensor_copy(out=dst, in_=src)
                    else:
                        nc.scalar.copy(out=dst, in_=src)

                def _emit_seg(pend):
                    for msk_p, jp in pend:
                        nc.tensor.matmul(out=outp[:], lhsT=msk_p[:],
                                         rhs=bslab[:, jp * 65:jp * 65 + 65],
                                         start=(jp == 0), stop=(jp == nj - 1),
                                         skip_group_check=True)

                slab3 = slab.rearrange("p (j c) -> p j c", c=65)
                bslab3 = bslab.rearrange("p (j c) -> p j c", c=65)
                pending = []
                for g in range(-(-nj // GB)):
                    w4 = min(GB, nj - g * GB)
                    w = w4 * CHUNK
                    c0 = g * GB * CHUNK
                    he = hps.tile([H, GB * CHUNK], f32, tag="hps")
                    ha = hps.tile([H, GB * CHUNK], f32, tag="hps")
                    nc.tensor.matmul(out=he[:, :w], lhsT=w1e[:],
                                     rhs=a1[:, c0:c0 + w], start=True, stop=False)
                    nc.tensor.matmul(out=he[:, :w], lhsT=w1be[:],
                                     rhs=a2[:, c0:c0 + w], start=False, stop=True)
                    nc.tensor.matmul(out=ha[:, :w], lhsT=w1a[:],
                                     rhs=a1[:, c0:c0 + w], start=True, stop=False)
                    nc.tensor.matmul(out=ha[:, :w], lhsT=w1ba[:],
                                     rhs=a2[:, c0:c0 + w], start=False, stop=True)
                    hesb = hpool.tile([H, GB * CHUNK], dt2, tag="hesb")
                    hasb = hpool.tile([H, GB * CHUNK], dt2, tag="hasb")
                    nc.scalar.activation(out=hesb[:, :w], in_=he[:, :w],
                                         func=Act.Relu, bias=be1[:])
                    nc.vector.tensor_scalar(out=hasb[:, :w], in0=ha[:, :w],
                                            scalar1=ba1[:], scalar2=0.0,
                                            op0=Alu.add, op1=Alu.max)

                    # layer-2 matmuls for the whole group into one psum bank:
                    # chunk j4 occupies cols [j4*65, j4*65+65): uef | logit.
                    ps = ulps.tile([PT, GB * 65], f32, tag="ulps")
                    ps3 = ps.rearrange("p (j c) -> p j c", c=65)
                    for j4 in range(w4):
                        cc = j4 * CHUNK
                        nc.tensor.matmul(out=ps[:, j4 * 65:j4 * 65 + D],
                                         lhsT=hesb[:, cc:cc + CHUNK],
                                         rhs=w2e[:], start=True, stop=True)
                        nc.tensor.matmul(out=ps[:, j4 * 65 + D:j4 * 65 + D + 1],
                                         lhsT=hasb[:, cc:cc + CHUNK],
                                         rhs=w2a[:], start=True, stop=True)

                    # previous group's segment matmuls (hides the
                    # logit->exp->mask->seg cross-engine latency chain)
                    _emit_seg(pending)
                    pending = []

                    p = spool.tile([PT, GB], f32, tag="p")
                    nc.scalar.activation(out=p[:, :w4], in_=ps3[:, :w4, 64],
                                         func=Act.Exp, bias=ba2[:])
                    j0 = g * GB
                    if be2 is not None:
                        for j4 in range(w4):
                            nc.vector.tensor_tensor(
                                out=slab3[:, j0 + j4, 0:D],
                                in0=ps3[:, j4, 0:D], in1=be2[:], op=Alu.add)
                            if SEG_BF16:
                                nc.scalar.copy(out=bslab3[:, j0 + j4, 0:D],
                                               in_=slab3[:, j0 + j4, 0:D])
                    else:
                        _copy(slab3[:, j0:j0 + w4, 0:D], ps3[:, :w4, 0:D],
                              g % 2 == 0)
                        if SEG_BF16:
                            _copy(bslab3[:, j0:j0 + w4, 0:D], ps3[:, :w4, 0:D],
                                  g % 2 == 1)
                    for j4 in range(w4):
                        j = j0 + j4
                        msk = mpool.tile([PT, PT], dts, tag="msk")
                        nc.vector.tensor_scalar(out=msk[:], in0=iota[:],
                                                scalar1=dl[:, j:j + 1],
                                                scalar2=p[:, j4:j4 + 1],
                                                op0=Alu.is_equal, op1=Alu.mult)
                        pending.append((msk, j))
                _emit_seg(pending)

                uef_view = uefd[o:o + ncols, :].rearrange(
                    "(j p) f -> p j f", p=PT)
                slab_view = slab.rearrange("p (j c) -> p j c", c=65)[:, :nj, 0:D]
                nc.sync.dma_start(out=uef_view, in_=slab_view)

                # ---- node phase ----
                r = spool.tile([PT, 1], f32, tag="r")
                nc.vector.tensor_scalar(out=r[:], in0=outp[:, D:D + 1],
                                        scalar1=1e-30, scalar2=None, op0=Alu.max)
                nc.vector.reciprocal(out=r[:], in_=r[:])
                nc.vector.tensor_scalar(out=r[:], in0=r[:],
                                        scalar1=dl[:, MAXJ:MAXJ + 1],
                                        scalar2=None, op0=Alu.mult)
                agg = spool.tile([PT, D], dtn, tag="agg")
                nc.vector.tensor_scalar(out=agg[:], in0=outp[:, 0:D],
                                        scalar1=r[:], scalar2=None, op0=Alu.mult)
                tp = nps.tile([D, PT], dtn, tag="nps")
                nc.tensor.transpose(tp[:], agg[:], idn[:])
                nin = npool.tile([2 * D, PT], dtn, tag="nin")
                nc.scalar.copy(out=nin[0:D, :], in_=tp[:])
                nc.sync.dma_start(out=nin[D:2 * D, :],
                                  in_=nftd[:, t * PT:(t + 1) * PT])
                hn_ps = nps.tile([H, PT], f32, tag="nps")
                nc.tensor.matmul(out=hn_ps[:], lhsT=wn1[:], rhs=nin[:],
                                 start=True, stop=True)
                hn = npool.tile([H, PT], dtn, tag="hn")
                nc.scalar.activation(out=hn[:], in_=hn_ps[:],
                                     func=Act.Relu, bias=bn1[:])
                unf_ps = nps.tile([PT, D], f32, tag="nps")
                nc.tensor.matmul(out=unf_ps[:], lhsT=hn[:], rhs=wn2[:],
                                 start=True, stop=True)
                unf_sb = npool.tile([PT, D], f32, tag="unfsb")
                if bn2 is not None:
                    nc.vector.tensor_tensor(out=unf_sb[:], in0=unf_ps[:],
                                            in1=bn2[:], op=Alu.add)
                else:
                    nc.vector.tensor_copy(out=unf_sb[:], in_=unf_ps[:])
                nc.sync.dma_start(out=unfd[t * PT:(t + 1) * PT, :], in_=unf_sb[:])

    nc.compile()
    return nc


# --------------------------------------------------------------------------
# entry point
# --------------------------------------------------------------------------

def kernel(nf, ef, We1, be1, We2, be2, Wa1, ba1, Wa2, ba2,
           Wn1, bn1, Wn2, bn2, src, dst):
    import ml_dtypes
    from concourse.bass_utils import run_bass_kernel_spmd

    nf = np.ascontiguousarray(np.asarray(nf, np.float32))
    ef = np.ascontiguousarray(np.asarray(ef, np.float32))
    meta = _prep(nf, ef, np.asarray(src), np.asarray(dst))

    has_be2 = bool(np.any(np.asarray(be2)))
    has_bn2 = bool(np.any(np.asarray(bn2)))

    key = (meta["E_pad"], meta["MAXJ"], tuple(meta["n_ch"].tolist()),
           has_be2, has_bn2, STAGE1_BF16, L2_BF16, SEG_BF16, NODE_BF16)
    if key not in _PROG_CACHE:
        _PROG_CACHE[key] = _build(meta, has_be2, has_bn2)
    nc = _PROG_CACHE[key]

    bfc = lambda a: np.ascontiguousarray(np.asarray(a, np.float32)).astype(
        ml_dtypes.bfloat16)
    f32 = lambda a: np.ascontiguousarray(np.asarray(a, np.float32))
    cast1 = bfc if STAGE1_BF16 else f32
    cast2 = bfc if L2_BF16 else f32
    casts = bfc if SEG_BF16 else f32
    castn = bfc if NODE_BF16 else f32

    shared = {
        "w1e": cast1(We1[:2 * D]), "w1be": cast1(We1[2 * D:]),
        "w1a": cast1(Wa1[:2 * D]), "w1ba": cast1(Wa1[2 * D:]),
        "w2e": cast2(We2), "w2a": cast2(Wa2),
        "wn1": castn(Wn1), "wn2": castn(Wn2),
        "be1": f32(be1), "ba1": f32(ba1), "bn1": f32(bn1),
        "ba2r": np.full(PT, np.float32(np.asarray(ba2).reshape(-1)[0])),
        "be2r": np.broadcast_to(f32(be2), (PT, D)).copy(),
        "bn2r": np.broadcast_to(f32(bn2), (PT, D)).copy(),
        "iota": casts(np.broadcast_to(np.arange(PT, dtype=np.float32),
                                      (PT, PT))),
        "idn": castn(np.eye(PT, dtype=np.float32)),
    }
    in_maps = []
    for c in range(NCORES):
        m = dict(shared)
        m["a1"] = cast1(meta["A1"][c])
        m["a2"] = cast1(meta["A2"][c])
        m["dl"] = meta["DL"][c]
        m["nft"] = castn(meta["NFT"][c])
        in_maps.append(m)

    res = run_bass_kernel_spmd(nc, in_maps, core_ids=list(range(NCORES)))
    global _LAST_RUN
    _LAST_RUN = res

    N, E, NPER, NT = meta["N"], meta["E"], meta["NPER"], meta["NT"]
    n_ch, offs, starts, K, perm = (meta["n_ch"], meta["offs"],
                                   meta["starts"], meta["K"], meta["perm"])
    unf = np.empty((N, D), np.float32)
    uef = np.empty((E, D), np.float32)
    for c in range(NCORES):
        lo, hi = c * NPER, min((c + 1) * NPER, N)
        unf[lo:hi] = res.results[c]["unf_out"][:hi - lo]
        uo = res.results[c]["uef_out"]
        for t in range(NT):
            g = c * NT + t
            s0, cnt = starts[g], K[c, t]
            if cnt:
                o = offs[t]
                uef[perm[s0:s0 + cnt]] = uo[o:o + cnt]
    return unf, uef


# revision 29
# speedup vs baseline: 3.3318x; 1.0011x over previous
"""Trainium2 Bass kernel for an attention MPNN layer (edge MLP + segment
softmax + scatter-mean + node MLP), distributed over 8 NeuronCores.

Strategy: host sorts edges by destination node and partitions BOTH the nodes
and their incoming edges across the 8 cores (node range [c*N/8,(c+1)*N/8) and
every edge pointing into it live on core c). Each core is then fully
independent -- no collectives. Within a core, nodes are processed in tiles of
128; a tile's incoming edges stream through the edge MLPs in chunks of 128,
and the segment softmax/mean reduction is realised as a mask matmul
(mask[e, n] = p_e * [dst_e == n]) accumulated in PSUM across the tile's
chunks. All irregularity (gather of nf[src]/nf[dst], sort, ragged segment
boundaries) is resolved on the host, so the device only executes dense
DMAs and matmuls.
"""

import math

import numpy as np

D = 64
H = 128
NCORES = 8
PT = 128  # nodes per tile (partition dim)
CHUNK = 128  # edges per sub-chunk
GB = 4  # sub-chunks per h-stage group (moving-dim 512)

# dtype config: stage-1 stream (edge features + layer-1 weights), layer-2
# (h activations + We2/Wa2), and the segment mask matmul. fp32 PSUM
# accumulation and fp32 softmax/normalization/node-MLP throughout.
STAGE1_BF16 = True
L2_BF16 = True
SEG_BF16 = True
NODE_BF16 = True

_PROG_CACHE = {}


# --------------------------------------------------------------------------
# host-side preprocessing
# --------------------------------------------------------------------------

def _prep(nf, ef, src, dst):
    N, E = nf.shape[0], ef.shape[0]
    NPER = -(-N // NCORES)
    NT = -(-NPER // PT)

    src = np.ascontiguousarray(src).astype(np.int64, copy=False)
    dst = np.ascontiguousarray(dst).astype(np.int64, copy=False)

    perm = np.argsort(dst, kind="stable")
    dsts = dst[perm]
    srcs = src[perm]
    core_e = dsts // NPER
    loc = dsts - core_e * NPER
    tloc = loc // PT
    gid = core_e * NT + tloc  # nondecreasing
    starts = np.searchsorted(gid, np.arange(NCORES * NT + 1), side="left")
    K = np.diff(starts).reshape(NCORES, NT)

    n_ch = np.maximum(1, -(-K.max(axis=0) // CHUNK)).astype(np.int64)  # [NT]
    offs = np.zeros(NT + 1, np.int64)
    offs[1:] = np.cumsum(n_ch) * CHUNK
    E_pad = int(offs[-1])
    MAXJ = int(n_ch.max())

    efs = ef[perm]
    nfss = nf[srcs]
    nfds = nf[dsts]
    deg = np.bincount(dst, minlength=N).astype(np.float64)
    invdeg = (1.0 / np.maximum(deg, 1.0)).astype(np.float32)

    A1 = np.zeros((NCORES, 2 * D, E_pad), np.float32)
    A2 = np.zeros((NCORES, D, E_pad), np.float32)
    DL = np.full((NCORES, NT, PT, MAXJ + 1), -1.0, np.float32)
    NFT = np.zeros((NCORES, D, NT * PT), np.float32)

    for c in range(NCORES):
        for t in range(NT):
            g = c * NT + t
            s0, cnt = starts[g], K[c, t]
            o = offs[t]
            if cnt:
                A1[c, :D, o:o + cnt] = efs[s0:s0 + cnt].T
                A1[c, D:, o:o + cnt] = nfss[s0:s0 + cnt].T
                A2[c, :, o:o + cnt] = nfds[s0:s0 + cnt].T
                pad = np.full(n_ch[t] * CHUNK, -1.0, np.float32)
                pad[:cnt] = (loc[s0:s0 + cnt] - t * PT).astype(np.float32)
                DL[c, t, :, :n_ch[t]] = pad.reshape(n_ch[t], CHUNK).T
        lo, hi = c * NPER, min((c + 1) * NPER, N)
        NFT[c, :, :hi - lo] = nf[lo:hi].T
        ipad = np.ones(NT * PT, np.float32)
        ipad[:hi - lo] = invdeg[lo:hi]
        DL[c, :, :, MAXJ] = ipad.reshape(NT, PT)

    return dict(N=N, E=E, NPER=NPER, NT=NT, E_pad=E_pad, MAXJ=MAXJ,
                n_ch=n_ch, offs=offs, starts=starts, K=K, perm=perm,
                A1=A1, A2=A2, DL=DL, NFT=NFT)


# --------------------------------------------------------------------------
# device program
# --------------------------------------------------------------------------

def _build(meta, has_be2, has_bn2):
    import concourse.bass as bass
    import concourse.tile as tile
    from concourse import bacc, mybir

    f32 = mybir.dt.float32
    bf16 = mybir.dt.bfloat16
    dt1 = bf16 if STAGE1_BF16 else f32
    dt2 = bf16 if L2_BF16 else f32
    dts = bf16 if SEG_BF16 else f32
    dtn = bf16 if NODE_BF16 else f32
    Alu = mybir.AluOpType
    Act = mybir.ActivationFunctionType

    NT, MAXJ, E_pad = meta["NT"], meta["MAXJ"], meta["E_pad"]
    n_ch = meta["n_ch"]

    nc = bacc.Bacc("TRN2", target_bir_lowering=False, debug=False,
                   enable_asserts=False, num_devices=NCORES)

    a1d = nc.dram_tensor("a1", [2 * D, E_pad], dt1, kind="ExternalInput").ap()
    a2d = nc.dram_tensor("a2", [D, E_pad], dt1, kind="ExternalInput").ap()
    dld = nc.dram_tensor("dl", [NT, PT, MAXJ + 1], f32, kind="ExternalInput").ap()
    nftd = nc.dram_tensor("nft", [D, NT * PT], dtn, kind="ExternalInput").ap()
    w1ed = nc.dram_tensor("w1e", [2 * D, H], dt1, kind="ExternalInput").ap()
    w1bed = nc.dram_tensor("w1be", [D, H], dt1, kind="ExternalInput").ap()
    w1ad = nc.dram_tensor("w1a", [2 * D, H], dt1, kind="ExternalInput").ap()
    w1bad = nc.dram_tensor("w1ba", [D, H], dt1, kind="ExternalInput").ap()
    w2ed = nc.dram_tensor("w2e", [H, D], dt2, kind="ExternalInput").ap()
    w2ad = nc.dram_tensor("w2a", [H, 1], dt2, kind="ExternalInput").ap()
    wn1d = nc.dram_tensor("wn1", [2 * D, H], dtn, kind="ExternalInput").ap()
    wn2d = nc.dram_tensor("wn2", [H, D], dtn, kind="ExternalInput").ap()
    be1d = nc.dram_tensor("be1", [H], f32, kind="ExternalInput").ap()
    ba1d = nc.dram_tensor("ba1", [H], f32, kind="ExternalInput").ap()
    bn1d = nc.dram_tensor("bn1", [H], f32, kind="ExternalInput").ap()
    ba2d = nc.dram_tensor("ba2r", [PT], f32, kind="ExternalInput").ap()
    be2d = nc.dram_tensor("be2r", [PT, D], f32, kind="ExternalInput").ap()
    bn2d = nc.dram_tensor("bn2r", [PT, D], f32, kind="ExternalInput").ap()
    iotad = nc.dram_tensor("iota", [PT, PT], dts, kind="ExternalInput").ap()
    idnd = nc.dram_tensor("idn", [PT, PT], dtn, kind="ExternalInput").ap()

    uefd = nc.dram_tensor("uef_out", [E_pad, D], f32, kind="ExternalOutput").ap()
    unfd = nc.dram_tensor("unf_out", [NT * PT, D], f32, kind="ExternalOutput").ap()

    # persistent uef slabs (double buffered across node tiles); col 64 of each
    # 65-wide chunk strip holds the constant 1.0 used to segment-sum p. The
    # fp32 slabs feed the uef output DMA; when SEG_BF16 a parallel bf16 pair
    # feeds the segment matmul.
    slabs = [nc.alloc_sbuf_tensor(f"slab{i}", [PT, MAXJ * 65], f32).ap()
             for i in range(2)]
    if SEG_BF16:
        bslabs = [nc.alloc_sbuf_tensor(f"bslab{i}", [PT, MAXJ * 65], dts).ap()
                  for i in range(2)]
    else:
        bslabs = slabs

    with tile.TileContext(nc) as tc:
        with tc.tile_pool(name="const", bufs=1) as cpool, \
             tc.tile_pool(name="a1p", bufs=2) as a1pool, \
             tc.tile_pool(name="a2p", bufs=2) as a2pool, \
             tc.tile_pool(name="dlp", bufs=2) as dlpool, \
             tc.tile_pool(name="hsb", bufs=4) as hpool, \
             tc.tile_pool(name="small", bufs=4) as spool, \
             tc.tile_pool(name="msk", bufs=10) as mpool, \
             tc.tile_pool(name="node", bufs=2) as npool, \
             tc.tile_pool(name="hps", bufs=2, space="PSUM") as hps, \
             tc.tile_pool(name="ulps", bufs=2, space="PSUM") as ulps, \
             tc.tile_pool(name="outps", bufs=2, space="PSUM") as outps, \
             tc.tile_pool(name="nps", bufs=2, space="PSUM") as nps:

            w1e = cpool.tile_from(w1ed)
            w1be = cpool.tile_from(w1bed)
            w1a = cpool.tile_from(w1ad)
            w1ba = cpool.tile_from(w1bad)
            w2e = cpool.tile_from(w2ed)
            w2a = cpool.tile_from(w2ad)
            wn1 = cpool.tile_from(wn1d)
            wn2 = cpool.tile_from(wn2d)
            be1 = cpool.tile_from(be1d[:, None])
            ba1 = cpool.tile_from(ba1d[:, None])
            bn1 = cpool.tile_from(bn1d[:, None])
            ba2 = cpool.tile_from(ba2d[:, None])
            iota = cpool.tile_from(iotad)
            idn = cpool.tile_from(idnd)
            be2 = cpool.tile_from(be2d) if has_be2 else None
            bn2 = cpool.tile_from(bn2d) if has_bn2 else None

            for s in bslabs:
                ones = s.rearrange("p (j c) -> p j c", c=65)[:, :, 64:65]
                nc.vector.memset(ones, 1.0)

            deferred_node = [None]

            def _flush_node():
                if deferred_node[0] is not None:
                    deferred_node[0]()
                    deferred_node[0] = None

            for t in range(NT):
                nj = int(n_ch[t])
                o = int(meta["offs"][t])
                ncols = nj * CHUNK
                slab = slabs[t % 2]
                bslab = bslabs[t % 2]

                a1 = a1pool.tile([2 * D, MAXJ * CHUNK], dt1, tag="a1")
                a2 = a2pool.tile([D, MAXJ * CHUNK], dt1, tag="a2")
                dl = dlpool.tile([PT, MAXJ + 1], f32, tag="dl")
                nc.sync.dma_start(out=a1[:, :ncols], in_=a1d[:, o:o + ncols])
                nc.sync.dma_start(out=a2[:, :ncols], in_=a2d[:, o:o + ncols])
                nc.sync.dma_start(out=dl[:], in_=dld[t])

                outp = outps.tile([PT, 65], f32, tag="outp")

                def _copy(dst, src, on_vector):
                    if on_vector:
                        nc.vector.tensor_copy(out=dst, in_=src)
                    else:
                        nc.scalar.copy(out=dst, in_=src)

                def _emit_seg(pend, outp=outp, bslab=bslab, nj=nj):
                    for msk_p, jp in pend:
                        nc.tensor.matmul(out=outp[:], lhsT=msk_p[:],
                                         rhs=bslab[:, jp * 65:jp * 65 + 65],
                                         start=(jp == 0), stop=(jp == nj - 1),
                                         skip_group_check=True)

                slab3 = slab.rearrange("p (j c) -> p j c", c=65)
                bslab3 = bslab.rearrange("p (j c) -> p j c", c=65)
                pending = []
                for g in range(-(-nj // GB)):
                    w4 = min(GB, nj - g * GB)
                    w = w4 * CHUNK
                    c0 = g * GB * CHUNK
                    he = hps.tile([H, GB * CHUNK], f32, tag="hps")
                    ha = hps.tile([H, GB * CHUNK], f32, tag="hps")
                    nc.tensor.matmul(out=he[:, :w], lhsT=w1e[:],
                                     rhs=a1[:, c0:c0 + w], start=True, stop=False)
                    nc.tensor.matmul(out=he[:, :w], lhsT=w1be[:],
                                     rhs=a2[:, c0:c0 + w], start=False, stop=True)
                    nc.tensor.matmul(out=ha[:, :w], lhsT=w1a[:],
                                     rhs=a1[:, c0:c0 + w], start=True, stop=False)
                    nc.tensor.matmul(out=ha[:, :w], lhsT=w1ba[:],
                                     rhs=a2[:, c0:c0 + w], start=False, stop=True)
                    hesb = hpool.tile([H, GB * CHUNK], dt2, tag="hesb")
                    hasb = hpool.tile([H, GB * CHUNK], dt2, tag="hasb")
                    nc.scalar.activation(out=hesb[:, :w], in_=he[:, :w],
                                         func=Act.Relu, bias=be1[:])
                    nc.vector.tensor_scalar(out=hasb[:, :w], in0=ha[:, :w],
                                            scalar1=ba1[:], scalar2=0.0,
                                            op0=Alu.add, op1=Alu.max)

                    # layer-2 matmuls for the whole group into one psum bank:
                    # chunk j4 occupies cols [j4*65, j4*65+65): uef | logit.
                    ps = ulps.tile([PT, GB * 65], f32, tag="ulps")
                    ps3 = ps.rearrange("p (j c) -> p j c", c=65)
                    for j4 in range(w4):
                        cc = j4 * CHUNK
                        nc.tensor.matmul(out=ps[:, j4 * 65:j4 * 65 + D],
                                         lhsT=hesb[:, cc:cc + CHUNK],
                                         rhs=w2e[:], start=True, stop=True)
                        nc.tensor.matmul(out=ps[:, j4 * 65 + D:j4 * 65 + D + 1],
                                         lhsT=hasb[:, cc:cc + CHUNK],
                                         rhs=w2a[:], start=True, stop=True)

                    # previous group's segment matmuls (hides the
                    # logit->exp->mask->seg cross-engine latency chain)
                    _emit_seg(pending)
                    pending = []
                    if g == 0:
                        # previous tile's node phase: all inputs long ready,
                        # keeps PE from stalling on cross-engine chains.
                        _flush_node()

                    p = spool.tile([PT, GB], f32, tag="p")
                    nc.scalar.activation(out=p[:, :w4], in_=ps3[:, :w4, 64],
                                         func=Act.Exp, bias=ba2[:])
                    j0 = g * GB
                    if be2 is not None:
                        for j4 in range(w4):
                            nc.vector.tensor_tensor(
                                out=slab3[:, j0 + j4, 0:D],
                                in0=ps3[:, j4, 0:D], in1=be2[:], op=Alu.add)
                            if SEG_BF16:
                                nc.scalar.copy(out=bslab3[:, j0 + j4, 0:D],
                                               in_=slab3[:, j0 + j4, 0:D])
                    else:
                        _copy(slab3[:, j0:j0 + w4, 0:D], ps3[:, :w4, 0:D],
                              g % 2 == 0)
                        if SEG_BF16:
                            _copy(bslab3[:, j0:j0 + w4, 0:D], ps3[:, :w4, 0:D],
                                  g % 2 == 1)
                    for j4 in range(w4):
                        j = j0 + j4
                        msk = mpool.tile([PT, PT], dts, tag="msk")
                        nc.vector.tensor_scalar(out=msk[:], in0=iota[:],
                                                scalar1=dl[:, j:j + 1],
                                                scalar2=p[:, j4:j4 + 1],
                                                op0=Alu.is_equal, op1=Alu.mult)
                        pending.append((msk, j))
                _emit_seg(pending)

                uef_view = uefd[o:o + ncols, :].rearrange(
                    "(j p) f -> p j f", p=PT)
                slab_view = slab.rearrange("p (j c) -> p j c", c=65)[:, :nj, 0:D]
                nc.sync.dma_start(out=uef_view, in_=slab_view)

                # segment-mean normalization, emitted now (DVE work while the
                # next tile streams); the matmul part of the node MLP is
                # deferred into the next tile's instruction stream.
                r = spool.tile([PT, 1], f32, tag="r")
                nc.vector.tensor_scalar(out=r[:], in0=outp[:, D:D + 1],
                                        scalar1=1e-30, scalar2=None, op0=Alu.max)
                nc.vector.reciprocal(out=r[:], in_=r[:])
                nc.vector.tensor_scalar(out=r[:], in0=r[:],
                                        scalar1=dl[:, MAXJ:MAXJ + 1],
                                        scalar2=None, op0=Alu.mult)
                agg = spool.tile([PT, D], dtn, tag="agg")
                nc.vector.tensor_scalar(out=agg[:], in0=outp[:, 0:D],
                                        scalar1=r[:], scalar2=None, op0=Alu.mult)

                def _node(t=t, agg=agg):
                    tp = nps.tile([D, PT], dtn, tag="nps")
                    nc.tensor.transpose(tp[:], agg[:], idn[:])
                    nin = npool.tile([2 * D, PT], dtn, tag="nin")
                    nc.scalar.copy(out=nin[0:D, :], in_=tp[:])
                    nc.sync.dma_start(out=nin[D:2 * D, :],
                                      in_=nftd[:, t * PT:(t + 1) * PT])
                    hn_ps = nps.tile([H, PT], f32, tag="nps")
                    nc.tensor.matmul(out=hn_ps[:], lhsT=wn1[:], rhs=nin[:],
                                     start=True, stop=True)
                    hn = npool.tile([H, PT], dtn, tag="hn")
                    nc.scalar.activation(out=hn[:], in_=hn_ps[:],
                                         func=Act.Relu, bias=bn1[:])
                    unf_ps = nps.tile([PT, D], f32, tag="nps")
                    nc.tensor.matmul(out=unf_ps[:], lhsT=hn[:], rhs=wn2[:],
                                     start=True, stop=True)
                    unf_sb = npool.tile([PT, D], f32, tag="unfsb")
                    if bn2 is not None:
                        nc.vector.tensor_tensor(out=unf_sb[:], in0=unf_ps[:],
                                                in1=bn2[:], op=Alu.add)
                    else:
                        nc.vector.tensor_copy(out=unf_sb[:], in_=unf_ps[:])
                    nc.sync.dma_start(out=unfd[t * PT:(t + 1) * PT, :],
                                      in_=unf_sb[:])

                deferred_node[0] = _node
            _flush_node()

    nc.compile()
    return nc


# --------------------------------------------------------------------------
# entry point
# --------------------------------------------------------------------------

def kernel(nf, ef, We1, be1, We2, be2, Wa1, ba1, Wa2, ba2,
           Wn1, bn1, Wn2, bn2, src, dst):
    import ml_dtypes
    from concourse.bass_utils import run_bass_kernel_spmd

    nf = np.ascontiguousarray(np.asarray(nf, np.float32))
    ef = np.ascontiguousarray(np.asarray(ef, np.float32))
    meta = _prep(nf, ef, np.asarray(src), np.asarray(dst))

    has_be2 = bool(np.any(np.asarray(be2)))
    has_bn2 = bool(np.any(np.asarray(bn2)))

    key = (meta["E_pad"], meta["MAXJ"], tuple(meta["n_ch"].tolist()),
           has_be2, has_bn2, STAGE1_BF16, L2_BF16, SEG_BF16, NODE_BF16)
    if key not in _PROG_CACHE:
        _PROG_CACHE[key] = _build(meta, has_be2, has_bn2)
    nc = _PROG_CACHE[key]

    bfc = lambda a: np.ascontiguousarray(np.asarray(a, np.float32)).astype(
        ml_dtypes.bfloat16)
    f32 = lambda a: np.ascontiguousarray(np.asarray(a, np.float32))
    cast1 = bfc if STAGE1_BF16 else f32
    cast2 = bfc if L2_BF16 else f32
    casts = bfc if SEG_BF16 else f32
    castn = bfc if NODE_BF16 else f32

    shared = {
        "w1e": cast1(We1[:2 * D]), "w1be": cast1(We1[2 * D:]),
        "w1a": cast1(Wa1[:2 * D]), "w1ba": cast1(Wa1[2 * D:]),
        "w2e": cast2(We2), "w2a": cast2(Wa2),
        "wn1": castn(Wn1), "wn2": castn(Wn2),
        "be1": f32(be1), "ba1": f32(ba1), "bn1": f32(bn1),
        "ba2r": np.full(PT, np.float32(np.asarray(ba2).reshape(-1)[0])),
        "be2r": np.broadcast_to(f32(be2), (PT, D)).copy(),
        "bn2r": np.broadcast_to(f32(bn2), (PT, D)).copy(),
        "iota": casts(np.broadcast_to(np.arange(PT, dtype=np.float32),
                                      (PT, PT))),
        "idn": castn(np.eye(PT, dtype=np.float32)),
    }
    in_maps = []
    for c in range(NCORES):
        m = dict(shared)
        m["a1"] = cast1(meta["A1"][c])
        m["a2"] = cast1(meta["A2"][c])
        m["dl"] = meta["DL"][c]
        m["nft"] = castn(meta["NFT"][c])
        in_maps.append(m)

    res = run_bass_kernel_spmd(nc, in_maps, core_ids=list(range(NCORES)))
    global _LAST_RUN
    _LAST_RUN = res

    N, E, NPER, NT = meta["N"], meta["E"], meta["NPER"], meta["NT"]
    n_ch, offs, starts, K, perm = (meta["n_ch"], meta["offs"],
                                   meta["starts"], meta["K"], meta["perm"])
    unf = np.empty((N, D), np.float32)
    uef = np.empty((E, D), np.float32)
    for c in range(NCORES):
        lo, hi = c * NPER, min((c + 1) * NPER, N)
        unf[lo:hi] = res.results[c]["unf_out"][:hi - lo]
        uo = res.results[c]["uef_out"]
        for t in range(NT):
            g = c * NT + t
            s0, cnt = starts[g], K[c, t]
            if cnt:
                o = offs[t]
                uef[perm[s0:s0 + cnt]] = uo[o:o + cnt]
    return unf, uef


# revision 30
# speedup vs baseline: 3.3337x; 1.0006x over previous
"""Trainium2 Bass kernel for an attention MPNN layer (edge MLP + segment
softmax + scatter-mean + node MLP), distributed over 8 NeuronCores.

Strategy: host sorts edges by destination node and partitions BOTH the nodes
and their incoming edges across the 8 cores (node range [c*N/8,(c+1)*N/8) and
every edge pointing into it live on core c). Each core is then fully
independent -- no collectives. Within a core, nodes are processed in tiles of
128; a tile's incoming edges stream through the edge MLPs in chunks of 128,
and the segment softmax/mean reduction is realised as a mask matmul
(mask[e, n] = p_e * [dst_e == n]) accumulated in PSUM across the tile's
chunks. All irregularity (gather of nf[src]/nf[dst], sort, ragged segment
boundaries) is resolved on the host, so the device only executes dense
DMAs and matmuls.
"""

import math

import numpy as np

D = 64
H = 128
NCORES = 8
PT = 128  # nodes per tile (partition dim)
CHUNK = 128  # edges per sub-chunk
GB = 4  # sub-chunks per h-stage group (moving-dim 512)

# dtype config: stage-1 stream (edge features + layer-1 weights), layer-2
# (h activations + We2/Wa2), and the segment mask matmul. fp32 PSUM
# accumulation and fp32 softmax/normalization/node-MLP throughout.
STAGE1_BF16 = True
L2_BF16 = True
SEG_BF16 = True
NODE_BF16 = True

_PROG_CACHE = {}


# --------------------------------------------------------------------------
# host-side preprocessing
# --------------------------------------------------------------------------

def _prep(nf, ef, src, dst):
    N, E = nf.shape[0], ef.shape[0]
    NPER = -(-N // NCORES)
    NT = -(-NPER // PT)

    src = np.ascontiguousarray(src).astype(np.int64, copy=False)
    dst = np.ascontiguousarray(dst).astype(np.int64, copy=False)

    perm = np.argsort(dst, kind="stable")
    dsts = dst[perm]
    srcs = src[perm]
    core_e = dsts // NPER
    loc = dsts - core_e * NPER
    tloc = loc // PT
    gid = core_e * NT + tloc  # nondecreasing
    starts = np.searchsorted(gid, np.arange(NCORES * NT + 1), side="left")
    K = np.diff(starts).reshape(NCORES, NT)

    n_ch = np.maximum(1, -(-K.max(axis=0) // CHUNK)).astype(np.int64)  # [NT]
    offs = np.zeros(NT + 1, np.int64)
    offs[1:] = np.cumsum(n_ch) * CHUNK
    E_pad = int(offs[-1])
    MAXJ = int(n_ch.max())

    efs = ef[perm]
    nfss = nf[srcs]
    nfds = nf[dsts]
    deg = np.bincount(dst, minlength=N).astype(np.float64)
    invdeg = (1.0 / np.maximum(deg, 1.0)).astype(np.float32)

    A1 = np.zeros((NCORES, 2 * D, E_pad), np.float32)
    A2 = np.zeros((NCORES, D, E_pad), np.float32)
    DL = np.full((NCORES, NT, PT, MAXJ + 1), -1.0, np.float32)
    NFT = np.zeros((NCORES, D, NT * PT), np.float32)

    for c in range(NCORES):
        for t in range(NT):
            g = c * NT + t
            s0, cnt = starts[g], K[c, t]
            o = offs[t]
            if cnt:
                A1[c, :D, o:o + cnt] = efs[s0:s0 + cnt].T
                A1[c, D:, o:o + cnt] = nfss[s0:s0 + cnt].T
                A2[c, :, o:o + cnt] = nfds[s0:s0 + cnt].T
                pad = np.full(n_ch[t] * CHUNK, -1.0, np.float32)
                pad[:cnt] = (loc[s0:s0 + cnt] - t * PT).astype(np.float32)
                DL[c, t, :, :n_ch[t]] = pad.reshape(n_ch[t], CHUNK).T
        lo, hi = c * NPER, min((c + 1) * NPER, N)
        NFT[c, :, :hi - lo] = nf[lo:hi].T
        ipad = np.ones(NT * PT, np.float32)
        ipad[:hi - lo] = invdeg[lo:hi]
        DL[c, :, :, MAXJ] = ipad.reshape(NT, PT)

    return dict(N=N, E=E, NPER=NPER, NT=NT, E_pad=E_pad, MAXJ=MAXJ,
                n_ch=n_ch, offs=offs, starts=starts, K=K, perm=perm,
                A1=A1, A2=A2, DL=DL, NFT=NFT)


# --------------------------------------------------------------------------
# device program
# --------------------------------------------------------------------------

def _build(meta, has_be2, has_bn2):
    import concourse.bass as bass
    import concourse.tile as tile
    from concourse import bacc, mybir

    f32 = mybir.dt.float32
    bf16 = mybir.dt.bfloat16
    dt1 = bf16 if STAGE1_BF16 else f32
    dt2 = bf16 if L2_BF16 else f32
    dts = bf16 if SEG_BF16 else f32
    dtn = bf16 if NODE_BF16 else f32
    Alu = mybir.AluOpType
    Act = mybir.ActivationFunctionType

    NT, MAXJ, E_pad = meta["NT"], meta["MAXJ"], meta["E_pad"]
    n_ch = meta["n_ch"]

    nc = bacc.Bacc("TRN2", target_bir_lowering=False, debug=False,
                   enable_asserts=False, num_devices=NCORES)

    a1d = nc.dram_tensor("a1", [2 * D, E_pad], dt1, kind="ExternalInput").ap()
    a2d = nc.dram_tensor("a2", [D, E_pad], dt1, kind="ExternalInput").ap()
    dld = nc.dram_tensor("dl", [NT, PT, MAXJ + 1], f32, kind="ExternalInput").ap()
    nftd = nc.dram_tensor("nft", [D, NT * PT], dtn, kind="ExternalInput").ap()
    w1ed = nc.dram_tensor("w1e", [2 * D, H], dt1, kind="ExternalInput").ap()
    w1bed = nc.dram_tensor("w1be", [D, H], dt1, kind="ExternalInput").ap()
    w1ad = nc.dram_tensor("w1a", [2 * D, H], dt1, kind="ExternalInput").ap()
    w1bad = nc.dram_tensor("w1ba", [D, H], dt1, kind="ExternalInput").ap()
    w2ed = nc.dram_tensor("w2e", [H, D], dt2, kind="ExternalInput").ap()
    w2ad = nc.dram_tensor("w2a", [H, 1], dt2, kind="ExternalInput").ap()
    wn1d = nc.dram_tensor("wn1", [2 * D, H], dtn, kind="ExternalInput").ap()
    wn2d = nc.dram_tensor("wn2", [H, D], dtn, kind="ExternalInput").ap()
    be1d = nc.dram_tensor("be1", [H], f32, kind="ExternalInput").ap()
    ba1d = nc.dram_tensor("ba1", [H], f32, kind="ExternalInput").ap()
    bn1d = nc.dram_tensor("bn1", [H], f32, kind="ExternalInput").ap()
    ba2d = nc.dram_tensor("ba2r", [PT], f32, kind="ExternalInput").ap()
    be2d = nc.dram_tensor("be2r", [PT, D], f32, kind="ExternalInput").ap()
    bn2d = nc.dram_tensor("bn2r", [PT, D], f32, kind="ExternalInput").ap()
    iotad = nc.dram_tensor("iota", [PT, PT], dts, kind="ExternalInput").ap()
    idnd = nc.dram_tensor("idn", [PT, PT], dtn, kind="ExternalInput").ap()

    uefd = nc.dram_tensor("uef_out", [E_pad, D], f32, kind="ExternalOutput").ap()
    unfd = nc.dram_tensor("unf_out", [NT * PT, D], f32, kind="ExternalOutput").ap()

    # persistent uef slabs (double buffered across node tiles); col 64 of each
    # 65-wide chunk strip holds the constant 1.0 used to segment-sum p. The
    # fp32 slabs feed the uef output DMA; when SEG_BF16 a parallel bf16 pair
    # feeds the segment matmul.
    slabs = [nc.alloc_sbuf_tensor(f"slab{i}", [PT, MAXJ * 65], f32).ap()
             for i in range(2)]
    if SEG_BF16:
        bslabs = [nc.alloc_sbuf_tensor(f"bslab{i}", [PT, MAXJ * 65], dts).ap()
                  for i in range(2)]
    else:
        bslabs = slabs

    with tile.TileContext(nc) as tc:
        with tc.tile_pool(name="const", bufs=1) as cpool, \
             tc.tile_pool(name="a1p", bufs=2) as a1pool, \
             tc.tile_pool(name="a2p", bufs=2) as a2pool, \
             tc.tile_pool(name="dlp", bufs=2) as dlpool, \
             tc.tile_pool(name="hsb", bufs=4) as hpool, \
             tc.tile_pool(name="small", bufs=4) as spool, \
             tc.tile_pool(name="msk", bufs=10) as mpool, \
             tc.tile_pool(name="node", bufs=2) as npool, \
             tc.tile_pool(name="hps", bufs=2, space="PSUM") as hps, \
             tc.tile_pool(name="ulps", bufs=2, space="PSUM") as ulps, \
             tc.tile_pool(name="outps", bufs=2, space="PSUM") as outps, \
             tc.tile_pool(name="nps", bufs=2, space="PSUM") as nps:

            w1e = cpool.tile_from(w1ed)
            w1be = cpool.tile_from(w1bed)
            w1a = cpool.tile_from(w1ad)
            w1ba = cpool.tile_from(w1bad)
            w2e = cpool.tile_from(w2ed)
            w2a = cpool.tile_from(w2ad)
            wn1 = cpool.tile_from(wn1d)
            wn2 = cpool.tile_from(wn2d)
            be1 = cpool.tile_from(be1d[:, None])
            ba1 = cpool.tile_from(ba1d[:, None])
            bn1 = cpool.tile_from(bn1d[:, None])
            ba2 = cpool.tile_from(ba2d[:, None])
            iota = cpool.tile_from(iotad)
            idn = cpool.tile_from(idnd)
            be2 = cpool.tile_from(be2d) if has_be2 else None
            bn2 = cpool.tile_from(bn2d) if has_bn2 else None

            for s in bslabs:
                ones = s.rearrange("p (j c) -> p j c", c=65)[:, :, 64:65]
                nc.vector.memset(ones, 1.0)

            deferred_node = [None]

            def _flush_node():
                if deferred_node[0] is not None:
                    deferred_node[0]()
                    deferred_node[0] = None

            for t in range(NT):
                nj = int(n_ch[t])
                o = int(meta["offs"][t])
                ncols = nj * CHUNK
                slab = slabs[t % 2]
                bslab = bslabs[t % 2]

                a1 = a1pool.tile([2 * D, MAXJ * CHUNK], dt1, tag="a1")
                a2 = a2pool.tile([D, MAXJ * CHUNK], dt1, tag="a2")
                dl = dlpool.tile([PT, MAXJ + 1], f32, tag="dl")
                nc.sync.dma_start(out=a1[:, :ncols], in_=a1d[:, o:o + ncols])
                nc.sync.dma_start(out=a2[:, :ncols], in_=a2d[:, o:o + ncols])
                nc.sync.dma_start(out=dl[:], in_=dld[t])

                outp = outps.tile([PT, 65], f32, tag="outp")

                def _copy(dst, src, on_vector):
                    if on_vector:
                        nc.vector.tensor_copy(out=dst, in_=src)
                    else:
                        nc.scalar.copy(out=dst, in_=src)

                def _emit_seg(pend, outp=outp, bslab=bslab, nj=nj):
                    for msk_p, jp in pend:
                        nc.tensor.matmul(out=outp[:], lhsT=msk_p[:],
                                         rhs=bslab[:, jp * 65:jp * 65 + 65],
                                         start=(jp == 0), stop=(jp == nj - 1),
                                         skip_group_check=True)

                slab3 = slab.rearrange("p (j c) -> p j c", c=65)
                bslab3 = bslab.rearrange("p (j c) -> p j c", c=65)
                pending = []
                for g in range(-(-nj // GB)):
                    w4 = min(GB, nj - g * GB)
                    w = w4 * CHUNK
                    c0 = g * GB * CHUNK
                    he = hps.tile([H, GB * CHUNK], f32, tag="hps")
                    ha = hps.tile([H, GB * CHUNK], f32, tag="hps")
                    nc.tensor.matmul(out=he[:, :w], lhsT=w1e[:],
                                     rhs=a1[:, c0:c0 + w], start=True, stop=False)
                    nc.tensor.matmul(out=he[:, :w], lhsT=w1be[:],
                                     rhs=a2[:, c0:c0 + w], start=False, stop=True)
                    nc.tensor.matmul(out=ha[:, :w], lhsT=w1a[:],
                                     rhs=a1[:, c0:c0 + w], start=True, stop=False)
                    nc.tensor.matmul(out=ha[:, :w], lhsT=w1ba[:],
                                     rhs=a2[:, c0:c0 + w], start=False, stop=True)
                    hesb = hpool.tile([H, GB * CHUNK], dt2, tag="hesb")
                    hasb = hpool.tile([H, GB * CHUNK], dt2, tag="hasb")
                    nc.scalar.activation(out=hesb[:, :w], in_=he[:, :w],
                                         func=Act.Relu, bias=be1[:])
                    nc.vector.tensor_scalar(out=hasb[:, :w], in0=ha[:, :w],
                                            scalar1=ba1[:], scalar2=0.0,
                                            op0=Alu.add, op1=Alu.max)

                    # previous group's segment matmuls go to the PE now: their
                    # inputs are long ready, so they fill the PE while the
                    # relus run (the layer-2 matmuls below must wait on them).
                    _emit_seg(pending)
                    pending = []
                    if g == 0:
                        # previous tile's node phase: all inputs long ready,
                        # keeps PE from stalling on cross-engine chains.
                        _flush_node()

                    # layer-2 matmuls for the whole group into one psum bank:
                    # chunk j4 occupies cols [j4*65, j4*65+65): uef | logit.
                    ps = ulps.tile([PT, GB * 65], f32, tag="ulps")
                    ps3 = ps.rearrange("p (j c) -> p j c", c=65)
                    for j4 in range(w4):
                        cc = j4 * CHUNK
                        nc.tensor.matmul(out=ps[:, j4 * 65:j4 * 65 + D],
                                         lhsT=hesb[:, cc:cc + CHUNK],
                                         rhs=w2e[:], start=True, stop=True)
                        nc.tensor.matmul(out=ps[:, j4 * 65 + D:j4 * 65 + D + 1],
                                         lhsT=hasb[:, cc:cc + CHUNK],
                                         rhs=w2a[:], start=True, stop=True)

                    p = spool.tile([PT, GB], f32, tag="p")
                    nc.scalar.activation(out=p[:, :w4], in_=ps3[:, :w4, 64],
                                         func=Act.Exp, bias=ba2[:])
                    j0 = g * GB
                    if be2 is not None:
                        for j4 in range(w4):
                            nc.vector.tensor_tensor(
                                out=slab3[:, j0 + j4, 0:D],
                                in0=ps3[:, j4, 0:D], in1=be2[:], op=Alu.add)
                            if SEG_BF16:
                                nc.scalar.copy(out=bslab3[:, j0 + j4, 0:D],
                                               in_=slab3[:, j0 + j4, 0:D])
                    else:
                        _copy(slab3[:, j0:j0 + w4, 0:D], ps3[:, :w4, 0:D],
                              g % 2 == 0)
                        if SEG_BF16:
                            _copy(bslab3[:, j0:j0 + w4, 0:D], ps3[:, :w4, 0:D],
                                  g % 2 == 1)
                    for j4 in range(w4):
                        j = j0 + j4
                        msk = mpool.tile([PT, PT], dts, tag="msk")
                        nc.vector.tensor_scalar(out=msk[:], in0=iota[:],
                                                scalar1=dl[:, j:j + 1],
                                                scalar2=p[:, j4:j4 + 1],
                                                op0=Alu.is_equal, op1=Alu.mult)
                        pending.append((msk, j))
                _emit_seg(pending)

                uef_view = uefd[o:o + ncols, :].rearrange(
                    "(j p) f -> p j f", p=PT)
                slab_view = slab.rearrange("p (j c) -> p j c", c=65)[:, :nj, 0:D]
                nc.sync.dma_start(out=uef_view, in_=slab_view)

                # segment-mean normalization, emitted now (DVE work while the
                # next tile streams); the matmul part of the node MLP is
                # deferred into the next tile's instruction stream.
                r = spool.tile([PT, 1], f32, tag="r")
                nc.vector.tensor_scalar(out=r[:], in0=outp[:, D:D + 1],
                                        scalar1=1e-30, scalar2=None, op0=Alu.max)
                nc.vector.reciprocal(out=r[:], in_=r[:])
                nc.vector.tensor_scalar(out=r[:], in0=r[:],
                                        scalar1=dl[:, MAXJ:MAXJ + 1],
                                        scalar2=None, op0=Alu.mult)
                agg = spool.tile([PT, D], dtn, tag="agg")
                nc.vector.tensor_scalar(out=agg[:], in0=outp[:, 0:D],
                                        scalar1=r[:], scalar2=None, op0=Alu.mult)

                def _node(t=t, agg=agg):
                    tp = nps.tile([D, PT], dtn, tag="nps")
                    nc.tensor.transpose(tp[:], agg[:], idn[:])
                    nin = npool.tile([2 * D, PT], dtn, tag="nin")
                    nc.scalar.copy(out=nin[0:D, :], in_=tp[:])
                    nc.sync.dma_start(out=nin[D:2 * D, :],
                                      in_=nftd[:, t * PT:(t + 1) * PT])
                    hn_ps = nps.tile([H, PT], f32, tag="nps")
                    nc.tensor.matmul(out=hn_ps[:], lhsT=wn1[:], rhs=nin[:],
                                     start=True, stop=True)
                    hn = npool.tile([H, PT], dtn, tag="hn")
                    nc.scalar.activation(out=hn[:], in_=hn_ps[:],
                                         func=Act.Relu, bias=bn1[:])
                    unf_ps = nps.tile([PT, D], f32, tag="nps")
                    nc.tensor.matmul(out=unf_ps[:], lhsT=hn[:], rhs=wn2[:],
                                     start=True, stop=True)
                    unf_sb = npool.tile([PT, D], f32, tag="unfsb")
                    if bn2 is not None:
                        nc.vector.tensor_tensor(out=unf_sb[:], in0=unf_ps[:],
                                                in1=bn2[:], op=Alu.add)
                    else:
                        nc.vector.tensor_copy(out=unf_sb[:], in_=unf_ps[:])
                    nc.sync.dma_start(out=unfd[t * PT:(t + 1) * PT, :],
                                      in_=unf_sb[:])

                deferred_node[0] = _node
            _flush_node()

    nc.compile()
    return nc


# --------------------------------------------------------------------------
# entry point
# --------------------------------------------------------------------------

def kernel(nf, ef, We1, be1, We2, be2, Wa1, ba1, Wa2, ba2,
           Wn1, bn1, Wn2, bn2, src, dst):
    import ml_dtypes
    from concourse.bass_utils import run_bass_kernel_spmd

    nf = np.ascontiguousarray(np.asarray(nf, np.float32))
    ef = np.ascontiguousarray(np.asarray(ef, np.float32))
    meta = _prep(nf, ef, np.asarray(src), np.asarray(dst))

    has_be2 = bool(np.any(np.asarray(be2)))
    has_bn2 = bool(np.any(np.asarray(bn2)))

    key = (meta["E_pad"], meta["MAXJ"], tuple(meta["n_ch"].tolist()),
           has_be2, has_bn2, STAGE1_BF16, L2_BF16, SEG_BF16, NODE_BF16)
    if key not in _PROG_CACHE:
        _PROG_CACHE[key] = _build(meta, has_be2, has_bn2)
    nc = _PROG_CACHE[key]

    bfc = lambda a: np.ascontiguousarray(np.asarray(a, np.float32)).astype(
        ml_dtypes.bfloat16)
    f32 = lambda a: np.ascontiguousarray(np.asarray(a, np.float32))
    cast1 = bfc if STAGE1_BF16 else f32
    cast2 = bfc if L2_BF16 else f32
    casts = bfc if SEG_BF16 else f32
    castn = bfc if NODE_BF16 else f32

    shared = {
        "w1e": cast1(We1[:2 * D]), "w1be": cast1(We1[2 * D:]),
        "w1a": cast1(Wa1[:2 * D]), "w1ba": cast1(Wa1[2 * D:]),
        "w2e": cast2(We2), "w2a": cast2(Wa2),
        "wn1": castn(Wn1), "wn2": castn(Wn2),
        "be1": f32(be1), "ba1": f32(ba1), "bn1": f32(bn1),
        "ba2r": np.full(PT, np.float32(np.asarray(ba2).reshape(-1)[0])),
        "be2r": np.broadcast_to(f32(be2), (PT, D)).copy(),
        "bn2r": np.broadcast_to(f32(bn2), (PT, D)).copy(),
        "iota": casts(np.broadcast_to(np.arange(PT, dtype=np.float32),
                                      (PT, PT))),
        "idn": castn(np.eye(PT, dtype=np.float32)),
    }
    in_maps = []
    for c in range(NCORES):
        m = dict(shared)
        m["a1"] = cast1(meta["A1"][c])
        m["a2"] = cast1(meta["A2"][c])
        m["dl"] = meta["DL"][c]
        m["nft"] = castn(meta["NFT"][c])
        in_maps.append(m)

    res = run_bass_kernel_spmd(nc, in_maps, core_ids=list(range(NCORES)))
    global _LAST_RUN
    _LAST_RUN = res

    N, E, NPER, NT = meta["N"], meta["E"], meta["NPER"], meta["NT"]
    n_ch, offs, starts, K, perm = (meta["n_ch"], meta["offs"],
                                   meta["starts"], meta["K"], meta["perm"])
    unf = np.empty((N, D), np.float32)
    uef = np.empty((E, D), np.float32)
    for c in range(NCORES):
        lo, hi = c * NPER, min((c + 1) * NPER, N)
        unf[lo:hi] = res.results[c]["unf_out"][:hi - lo]
        uo = res.results[c]["uef_out"]
        for t in range(NT):
            g = c * NT + t
            s0, cnt = starts[g], K[c, t]
            if cnt:
                o = offs[t]
                uef[perm[s0:s0 + cnt]] = uo[o:o + cnt]
    return unf, uef


# revision 32
# speedup vs baseline: 34.4747x; 10.3414x over previous
"""Trainium2 Bass kernel for an attention MPNN layer (edge MLP + segment
softmax + scatter-mean + node MLP), distributed over 8 NeuronCores.

Strategy: host sorts edges by destination node and partitions BOTH the nodes
and their incoming edges across the 8 cores (node range [c*N/8,(c+1)*N/8) and
every edge pointing into it live on core c). Each core is then fully
independent -- no collectives. Within a core, nodes are processed in tiles of
128; a tile's incoming edges stream through the edge MLPs in chunks of 128,
and the segment softmax/mean reduction is realised as a mask matmul
(mask[e, n] = p_e * [dst_e == n]) accumulated in PSUM across the tile's
chunks. All irregularity (gather of nf[src]/nf[dst], sort, ragged segment
boundaries) is resolved on the host, so the device only executes dense
DMAs and matmuls.
"""

import math

import numpy as np

D = 64
H = 128
NCORES = 8
PT = 128  # nodes per tile (partition dim)
CHUNK = 128  # edges per sub-chunk
GB = 4  # sub-chunks per h-stage group (moving-dim 512)

# dtype config: stage-1 stream (edge features + layer-1 weights), layer-2
# (h activations + We2/Wa2), and the segment mask matmul. fp32 PSUM
# accumulation and fp32 softmax/normalization/node-MLP throughout.
STAGE1_BF16 = True
L2_BF16 = True
SEG_BF16 = True
NODE_BF16 = True

_PROG_CACHE = {}


# --------------------------------------------------------------------------
# host-side preprocessing
# --------------------------------------------------------------------------

def _prep(nf, ef, src, dst):
    N, E = nf.shape[0], ef.shape[0]
    NPER = -(-N // NCORES)
    NT = -(-NPER // PT)

    src = np.ascontiguousarray(src).astype(np.int64, copy=False)
    dst = np.ascontiguousarray(dst).astype(np.int64, copy=False)

    perm = np.argsort(dst, kind="stable")
    dsts = dst[perm]
    srcs = src[perm]
    core_e = dsts // NPER
    loc = dsts - core_e * NPER
    tloc = loc // PT
    gid = core_e * NT + tloc  # nondecreasing
    starts = np.searchsorted(gid, np.arange(NCORES * NT + 1), side="left")
    K = np.diff(starts).reshape(NCORES, NT)

    n_ch = np.maximum(1, -(-K.max(axis=0) // CHUNK)).astype(np.int64)  # [NT]
    offs = np.zeros(NT + 1, np.int64)
    offs[1:] = np.cumsum(n_ch) * CHUNK
    E_pad = int(offs[-1])
    MAXJ = int(n_ch.max())

    efs = ef[perm]
    nfss = nf[srcs]
    nfds = nf[dsts]
    deg = np.bincount(dst, minlength=N).astype(np.float64)
    invdeg = (1.0 / np.maximum(deg, 1.0)).astype(np.float32)

    A1 = np.zeros((NCORES, 2 * D, E_pad), np.float32)
    A2 = np.zeros((NCORES, D, E_pad), np.float32)
    DL = np.full((NCORES, NT, PT, MAXJ + 1), -1.0, np.float32)
    NFT = np.zeros((NCORES, D, NT * PT), np.float32)

    for c in range(NCORES):
        for t in range(NT):
            g = c * NT + t
            s0, cnt = starts[g], K[c, t]
            o = offs[t]
            if cnt:
                A1[c, :D, o:o + cnt] = efs[s0:s0 + cnt].T
                A1[c, D:, o:o + cnt] = nfss[s0:s0 + cnt].T
                A2[c, :, o:o + cnt] = nfds[s0:s0 + cnt].T
                pad = np.full(n_ch[t] * CHUNK, -1.0, np.float32)
                pad[:cnt] = (loc[s0:s0 + cnt] - t * PT).astype(np.float32)
                DL[c, t, :, :n_ch[t]] = pad.reshape(n_ch[t], CHUNK).T
        lo, hi = c * NPER, min((c + 1) * NPER, N)
        NFT[c, :, :hi - lo] = nf[lo:hi].T
        ipad = np.ones(NT * PT, np.float32)
        ipad[:hi - lo] = invdeg[lo:hi]
        DL[c, :, :, MAXJ] = ipad.reshape(NT, PT)

    return dict(N=N, E=E, NPER=NPER, NT=NT, E_pad=E_pad, MAXJ=MAXJ,
                n_ch=n_ch, offs=offs, starts=starts, K=K, perm=perm,
                A1=A1, A2=A2, DL=DL, NFT=NFT)


# --------------------------------------------------------------------------
# device program
# --------------------------------------------------------------------------

def _build(meta, has_be2, has_bn2):
    import concourse.bass as bass
    import concourse.tile as tile
    from concourse import bacc, mybir

    f32 = mybir.dt.float32
    bf16 = mybir.dt.bfloat16
    dt1 = bf16 if STAGE1_BF16 else f32
    dt2 = bf16 if L2_BF16 else f32
    dts = bf16 if SEG_BF16 else f32
    dtn = bf16 if NODE_BF16 else f32
    Alu = mybir.AluOpType
    Act = mybir.ActivationFunctionType

    NT, MAXJ, E_pad = meta["NT"], meta["MAXJ"], meta["E_pad"]
    n_ch = meta["n_ch"]

    nc = bacc.Bacc("TRN2", target_bir_lowering=False, debug=False,
                   enable_asserts=False, num_devices=NCORES)

    a1d = nc.dram_tensor("a1", [2 * D, E_pad], dt1, kind="ExternalInput").ap()
    a2d = nc.dram_tensor("a2", [D, E_pad], dt1, kind="ExternalInput").ap()
    dld = nc.dram_tensor("dl", [NT, PT, MAXJ + 1], f32, kind="ExternalInput").ap()
    nftd = nc.dram_tensor("nft", [D, NT * PT], dtn, kind="ExternalInput").ap()
    w1ed = nc.dram_tensor("w1e", [2 * D, H], dt1, kind="ExternalInput").ap()
    w1bed = nc.dram_tensor("w1be", [D, H], dt1, kind="ExternalInput").ap()
    w1ad = nc.dram_tensor("w1a", [2 * D, H], dt1, kind="ExternalInput").ap()
    w1bad = nc.dram_tensor("w1ba", [D, H], dt1, kind="ExternalInput").ap()
    w2ed = nc.dram_tensor("w2e", [H, D], dt2, kind="ExternalInput").ap()
    w2ad = nc.dram_tensor("w2a", [H, 1], dt2, kind="ExternalInput").ap()
    wn1d = nc.dram_tensor("wn1", [2 * D, H], dtn, kind="ExternalInput").ap()
    wn2d = nc.dram_tensor("wn2", [H, D], dtn, kind="ExternalInput").ap()
    be1d = nc.dram_tensor("be1", [H], f32, kind="ExternalInput").ap()
    ba1d = nc.dram_tensor("ba1", [H], f32, kind="ExternalInput").ap()
    bn1d = nc.dram_tensor("bn1", [H], f32, kind="ExternalInput").ap()
    ba2d = nc.dram_tensor("ba2r", [PT], f32, kind="ExternalInput").ap()
    be2d = nc.dram_tensor("be2r", [PT, D], f32, kind="ExternalInput").ap()
    bn2d = nc.dram_tensor("bn2r", [PT, D], f32, kind="ExternalInput").ap()
    iotad = nc.dram_tensor("iota", [PT, PT], dts, kind="ExternalInput").ap()
    idnd = nc.dram_tensor("idn", [PT, PT], dtn, kind="ExternalInput").ap()

    uefd = nc.dram_tensor("uef_out", [E_pad, D], f32, kind="ExternalOutput").ap()
    unfd = nc.dram_tensor("unf_out", [NT * PT, D], f32, kind="ExternalOutput").ap()

    # persistent uef slabs (double buffered across node tiles). The fp32
    # slabs feed the uef output DMA; the bf16 pair feeds the segment matmul,
    # rows scaled by p with p itself in col 64 of each 65-wide strip.
    slabs = [nc.alloc_sbuf_tensor(f"slab{i}", [PT, MAXJ * 65], f32).ap()
             for i in range(2)]
    if SEG_BF16:
        bslabs = [nc.alloc_sbuf_tensor(f"bslab{i}", [PT, MAXJ * 65], dts).ap()
                  for i in range(2)]
    else:
        bslabs = slabs

    with tile.TileContext(nc) as tc:
        with tc.tile_pool(name="const", bufs=1) as cpool, \
             tc.tile_pool(name="a1p", bufs=2) as a1pool, \
             tc.tile_pool(name="a2p", bufs=2) as a2pool, \
             tc.tile_pool(name="dlp", bufs=2) as dlpool, \
             tc.tile_pool(name="hsb", bufs=4) as hpool, \
             tc.tile_pool(name="small", bufs=4) as spool, \
             tc.tile_pool(name="msk", bufs=24) as mpool, \
             tc.tile_pool(name="node", bufs=2) as npool, \
             tc.tile_pool(name="hps", bufs=3, space="PSUM") as hps, \
             tc.tile_pool(name="ulps", bufs=2, space="PSUM") as ulps, \
             tc.tile_pool(name="outps", bufs=1, space="PSUM") as outps, \
             tc.tile_pool(name="nps", bufs=2, space="PSUM") as nps:

            w1e = cpool.tile_from(w1ed)
            w1be = cpool.tile_from(w1bed)
            w1a = cpool.tile_from(w1ad)
            w1ba = cpool.tile_from(w1bad)
            w2e = cpool.tile_from(w2ed)
            w2a = cpool.tile_from(w2ad)
            wn1 = cpool.tile_from(wn1d)
            wn2 = cpool.tile_from(wn2d)
            be1 = cpool.tile_from(be1d[:, None])
            ba1 = cpool.tile_from(ba1d[:, None])
            bn1 = cpool.tile_from(bn1d[:, None])
            ba2 = cpool.tile_from(ba2d[:, None])
            iota = cpool.tile_from(iotad)
            idn = cpool.tile_from(idnd)
            be2 = cpool.tile_from(be2d) if has_be2 else None
            bn2 = cpool.tile_from(bn2d) if has_bn2 else None


            deferred_node = [None]

            def _flush_node():
                if deferred_node[0] is not None:
                    deferred_node[0]()
                    deferred_node[0] = None

            for t in range(NT):
                nj = int(n_ch[t])
                o = int(meta["offs"][t])
                ncols = nj * CHUNK
                slab = slabs[t % 2]
                bslab = bslabs[t % 2]

                a1 = a1pool.tile([2 * D, MAXJ * CHUNK], dt1, tag="a1")
                a2 = a2pool.tile([D, MAXJ * CHUNK], dt1, tag="a2")
                dl = dlpool.tile([PT, MAXJ + 1], f32, tag="dl")
                nc.sync.dma_start(out=a1[:, :ncols], in_=a1d[:, o:o + ncols])
                nc.sync.dma_start(out=a2[:, :ncols], in_=a2d[:, o:o + ncols])
                nc.sync.dma_start(out=dl[:], in_=dld[t])

                outp = outps.tile([PT, 65], f32, tag="outp")

                def _copy(dst, src, on_vector):
                    if on_vector:
                        nc.vector.tensor_copy(out=dst, in_=src)
                    else:
                        nc.scalar.copy(out=dst, in_=src)

                def _emit_seg(pend, outp=outp, bslab=bslab, nj=nj):
                    for msk_p, jp in pend:
                        nc.tensor.matmul(out=outp[:], lhsT=msk_p[:],
                                         rhs=bslab[:, jp * 65:jp * 65 + 65],
                                         start=(jp == 0), stop=(jp == nj - 1),
                                         skip_group_check=True)

                slab3 = slab.rearrange("p (j c) -> p j c", c=65)
                bslab3 = bslab.rearrange("p (j c) -> p j c", c=65)
                msks = []
                for j in range(nj):
                    msk = mpool.tile([PT, PT], dts, tag="msk")
                    nc.gpsimd.tensor_scalar(out=msk[:], in0=iota[:],
                                            scalar1=dl[:, j:j + 1],
                                            scalar2=None, op0=Alu.is_equal)
                    msks.append(msk)
                pending = []
                for g in range(-(-nj // GB)):
                    w4 = min(GB, nj - g * GB)
                    w = w4 * CHUNK
                    c0 = g * GB * CHUNK
                    he = hps.tile([H, GB * CHUNK], f32, tag="hps")
                    ha = hps.tile([H, GB * CHUNK], f32, tag="hps")
                    nc.tensor.matmul(out=he[:, :w], lhsT=w1e[:],
                                     rhs=a1[:, c0:c0 + w], start=True, stop=False)
                    nc.tensor.matmul(out=he[:, :w], lhsT=w1be[:],
                                     rhs=a2[:, c0:c0 + w], start=False, stop=True)
                    nc.tensor.matmul(out=ha[:, :w], lhsT=w1a[:],
                                     rhs=a1[:, c0:c0 + w], start=True, stop=False)
                    nc.tensor.matmul(out=ha[:, :w], lhsT=w1ba[:],
                                     rhs=a2[:, c0:c0 + w], start=False, stop=True)
                    hesb = hpool.tile([H, GB * CHUNK], dt2, tag="hesb")
                    hasb = hpool.tile([H, GB * CHUNK], dt2, tag="hasb")
                    nc.scalar.activation(out=hesb[:, :w], in_=he[:, :w],
                                         func=Act.Relu, bias=be1[:])
                    nc.vector.tensor_scalar(out=hasb[:, :w], in0=ha[:, :w],
                                            scalar1=ba1[:], scalar2=0.0,
                                            op0=Alu.add, op1=Alu.max)

                    # previous group's segment matmuls go to the PE now: their
                    # inputs are long ready, so they fill the PE while the
                    # relus run (the layer-2 matmuls below must wait on them).
                    _emit_seg(pending)
                    pending = []
                    if g == 0:
                        # previous tile's node phase: all inputs long ready,
                        # keeps PE from stalling on cross-engine chains.
                        _flush_node()

                    # layer-2 matmuls for the whole group into one psum bank:
                    # chunk j4 occupies cols [j4*65, j4*65+65): uef | logit.
                    ps = ulps.tile([PT, GB * 65], f32, tag="ulps")
                    ps3 = ps.rearrange("p (j c) -> p j c", c=65)
                    for j4 in range(w4):
                        cc = j4 * CHUNK
                        nc.tensor.matmul(out=ps[:, j4 * 65:j4 * 65 + D],
                                         lhsT=hesb[:, cc:cc + CHUNK],
                                         rhs=w2e[:], start=True, stop=True)
                        nc.tensor.matmul(out=ps[:, j4 * 65 + D:j4 * 65 + D + 1],
                                         lhsT=hasb[:, cc:cc + CHUNK],
                                         rhs=w2a[:], start=True, stop=True)

                    p = spool.tile([PT, GB], f32, tag="p")
                    nc.scalar.activation(out=p[:, :w4], in_=ps3[:, :w4, 64],
                                         func=Act.Exp, bias=ba2[:])
                    j0 = g * GB
                    if be2 is not None:
                        for j4 in range(w4):
                            nc.vector.tensor_tensor(
                                out=slab3[:, j0 + j4, 0:D],
                                in0=ps3[:, j4, 0:D], in1=be2[:], op=Alu.add)
                            nc.vector.tensor_scalar(
                                out=bslab3[:, j0 + j4, 0:D],
                                in0=slab3[:, j0 + j4, 0:D],
                                scalar1=p[:, j4:j4 + 1], scalar2=None,
                                op0=Alu.mult)
                    else:
                        _copy(slab3[:, j0:j0 + w4, 0:D], ps3[:, :w4, 0:D],
                              g % 2 == 0)
                        for j4 in range(w4):
                            nc.vector.tensor_scalar(
                                out=bslab3[:, j0 + j4, 0:D],
                                in0=ps3[:, j4, 0:D],
                                scalar1=p[:, j4:j4 + 1], scalar2=None,
                                op0=Alu.mult)
                    nc.vector.tensor_copy(out=bslab3[:, j0:j0 + w4, 64],
                                          in_=p[:, :w4])
                    for j4 in range(w4):
                        pending.append((msks[j0 + j4], j0 + j4))
                _emit_seg(pending)

                uef_view = uefd[o:o + ncols, :].rearrange(
                    "(j p) f -> p j f", p=PT)
                slab_view = slab.rearrange("p (j c) -> p j c", c=65)[:, :nj, 0:D]
                nc.sync.dma_start(out=uef_view, in_=slab_view)

                # segment-mean normalization, emitted now (DVE work while the
                # next tile streams); the matmul part of the node MLP is
                # deferred into the next tile's instruction stream.
                r = spool.tile([PT, 1], f32, tag="r")
                nc.vector.tensor_scalar(out=r[:], in0=outp[:, D:D + 1],
                                        scalar1=1e-30, scalar2=None, op0=Alu.max)
                nc.vector.reciprocal(out=r[:], in_=r[:])
                nc.vector.tensor_scalar(out=r[:], in0=r[:],
                                        scalar1=dl[:, MAXJ:MAXJ + 1],
                                        scalar2=None, op0=Alu.mult)
                agg = spool.tile([PT, D], dtn, tag="agg")
                nc.vector.tensor_scalar(out=agg[:], in0=outp[:, 0:D],
                                        scalar1=r[:], scalar2=None, op0=Alu.mult)

                def _node(t=t, agg=agg):
                    tp = nps.tile([D, PT], dtn, tag="nps")
                    nc.tensor.transpose(tp[:], agg[:], idn[:])
                    nin = npool.tile([2 * D, PT], dtn, tag="nin")
                    nc.scalar.copy(out=nin[0:D, :], in_=tp[:])
                    nc.sync.dma_start(out=nin[D:2 * D, :],
                                      in_=nftd[:, t * PT:(t + 1) * PT])
                    hn_ps = nps.tile([H, PT], f32, tag="nps")
                    nc.tensor.matmul(out=hn_ps[:], lhsT=wn1[:], rhs=nin[:],
                                     start=True, stop=True)
                    hn = npool.tile([H, PT], dtn, tag="hn")
                    nc.scalar.activation(out=hn[:], in_=hn_ps[:],
                                         func=Act.Relu, bias=bn1[:])
                    unf_ps = nps.tile([PT, D], f32, tag="nps")
                    nc.tensor.matmul(out=unf_ps[:], lhsT=hn[:], rhs=wn2[:],
                                     start=True, stop=True)
                    unf_sb = npool.tile([PT, D], f32, tag="unfsb")
                    if bn2 is not None:
                        nc.vector.tensor_tensor(out=unf_sb[:], in0=unf_ps[:],
                                                in1=bn2[:], op=Alu.add)
                    else:
                        nc.vector.tensor_copy(out=unf_sb[:], in_=unf_ps[:])
                    nc.sync.dma_start(out=unfd[t * PT:(t + 1) * PT, :],
                                      in_=unf_sb[:])

                deferred_node[0] = _node
            _flush_node()

    nc.compile()
    return nc


# --------------------------------------------------------------------------
# entry point
# --------------------------------------------------------------------------

def kernel(nf, ef, We1, be1, We2, be2, Wa1, ba1, Wa2, ba2,
           Wn1, bn1, Wn2, bn2, src, dst):
    import ml_dtypes
    from concourse.bass_utils import run_bass_kernel_spmd

    nf = np.ascontiguousarray(np.asarray(nf, np.float32))
    ef = np.ascontiguousarray(np.asarray(ef, np.float32))
    meta = _prep(nf, ef, np.asarray(src), np.asarray(dst))

    has_be2 = bool(np.any(np.asarray(be2)))
    has_bn2 = bool(np.any(np.asarray(bn2)))

    key = (meta["E_pad"], meta["MAXJ"], tuple(meta["n_ch"].tolist()),
           has_be2, has_bn2, STAGE1_BF16, L2_BF16, SEG_BF16, NODE_BF16)
    if key not in _PROG_CACHE:
        _PROG_CACHE[key] = _build(meta, has_be2, has_bn2)
    nc = _PROG_CACHE[key]

    bfc = lambda a: np.ascontiguousarray(np.asarray(a, np.float32)).astype(
        ml_dtypes.bfloat16)
    f32 = lambda a: np.ascontiguousarray(np.asarray(a, np.float32))
    cast1 = bfc if STAGE1_BF16 else f32
    cast2 = bfc if L2_BF16 else f32
    casts = bfc if SEG_BF16 else f32
    castn = bfc if NODE_BF16 else f32

    shared = {
        "w1e": cast1(We1[:2 * D]), "w1be": cast1(We1[2 * D:]),
        "w1a": cast1(Wa1[:2 * D]), "w1ba": cast1(Wa1[2 * D:]),
        "w2e": cast2(We2), "w2a": cast2(Wa2),
        "wn1": castn(Wn1), "wn2": castn(Wn2),
        "be1": f32(be1), "ba1": f32(ba1), "bn1": f32(bn1),
        "ba2r": np.full(PT, np.float32(np.asarray(ba2).reshape(-1)[0])),
        "be2r": np.broadcast_to(f32(be2), (PT, D)).copy(),
        "bn2r": np.broadcast_to(f32(bn2), (PT, D)).copy(),
        "iota": casts(np.broadcast_to(np.arange(PT, dtype=np.float32),
                                      (PT, PT))),
        "idn": castn(np.eye(PT, dtype=np.float32)),
    }
    in_maps = []
    for c in range(NCORES):
        m = dict(shared)
        m["a1"] = cast1(meta["A1"][c])
        m["a2"] = cast1(meta["A2"][c])
        m["dl"] = meta["DL"][c]
        m["nft"] = castn(meta["NFT"][c])
        in_maps.append(m)

    res = run_bass_kernel_spmd(nc, in_maps, core_ids=list(range(NCORES)))
    global _LAST_RUN
    _LAST_RUN = res

    N, E, NPER, NT = meta["N"], meta["E"], meta["NPER"], meta["NT"]
    n_ch, offs, starts, K, perm = (meta["n_ch"], meta["offs"],
                                   meta["starts"], meta["K"], meta["perm"])
    unf = np.empty((N, D), np.float32)
    uef = np.empty((E, D), np.float32)
    for c in range(NCORES):
        lo, hi = c * NPER, min((c + 1) * NPER, N)
        unf[lo:hi] = res.results[c]["unf_out"][:hi - lo]
        uo = res.results[c]["uef_out"]
        for t in range(NT):
            g = c * NT + t
            s0, cnt = starts[g], K[c, t]
            if cnt:
                o = offs[t]
                uef[perm[s0:s0 + cnt]] = uo[o:o + cnt]
    return unf, uef
